# revision 37
# baseline (speedup 1.0000x reference)
"""Trainium2 Bass kernel for ArccosHessianCalculator (v3).

Math: for each batch element b (z1, z2 are [B, D] with D = 128):
  a = 1/|z1|, bb = 1/|z2|, c = cos = <z1u, z2u>
  Each Hessian block H_k is rank-2 plus a diagonal term:
      H_k(b) = z1 * r0_k(b)^T + z2 * r1_k(b)^T + diag
  with the normalization / cosine factors folded into r0/r1:
      k=0 (H11): r0 = -3c*a^4*z1 + a^3 b*z2          r1 = a^3 b*z1
      k=1 (H12): r0 = a^3 b*z1                        r1 = -c*a^2 b^2*z1 + a b^3*z2
      k=2 (H22): r0 = a b^3*z2                        r1 = a b^3*z1 - 3c*b^4*z2

Device strategy (per core, batch shard of 512):
  - TensorE: one K=2 matmul per element, lhsT = [z1(b); z2(b)] ([2,128] fp16),
    rhs = [r0 | r1] blocks ([2, 384] fp16) -> PSUM f32 [128, 384]. Elements
    alternate PE quadrant rows ({0,1} / {32,33}) so LDWEIGHTS for element
    e+1 overlaps the stream of element e.
  - The rank-2 part goes to DRAM in its native layout as fp16:
    out_hw [128 i, B_SH*384 (b,k,j)] -- per-partition contiguous runs, so
    the output DMA runs at descriptor line rate; fp16 halves HBM traffic.
  - Final diagonals are computed in closed form per group ([128 b, 3*128 i])
    into a tiny f32 side tensor and spliced on the HOST (no predicated copy
    on device).
  - PSUM is organized as 4-bank super-tiles: 4 matmuls fill one, then ONE
    strided copy drains all 4 to fp16 SBUF, amortizing per-instruction
    overhead. Copies alternate DVE / ACT.
  - Stats run on GPSIMD for groups >= 1 (group 0 on DVE for a fast ramp);
    the R0/R1 builds stay on DVE (fp16 writes), sqrt on ACT, recip on DVE.
  - Batch rows are loaded interleaved (even elements -> partitions 0..63,
    odd -> 64..127) so per-chunk gathers are plain partition slices.
  - Host: one reshape/transpose view + diagonal stride-trick splice + cast.
"""

import numpy as np
from contextlib import ExitStack

import concourse.bass as bass
import concourse.tile as tile
from concourse import bacc, mybir
from concourse.bass_utils import run_bass_kernel_spmd

N_CORES = 8
B_FULL = 4096
D = 128
B_SH = B_FULL // N_CORES  # 512 batch elements per core
P = 128                   # SBUF partitions
KD = 3 * D                # 384: three H blocks side by side
F = 16                    # elements per gather partition-row
G = 32                    # elements per chunk
GROUPS = B_SH // P        # 4 stats groups of 128 elements
NCH = P // G              # 4 chunks per group
QB = 2                    # elements per PSUM super-tile (2 banks)

f32 = mybir.dt.float32
f16 = mybir.dt.float16
i8 = mybir.dt.int8

# interleaved element order within a group: partition p holds group element
# 2p (p < 64) or 2(p-64)+1 (p >= 64)
ELEM_OF_PART = np.concatenate([np.arange(0, P, 2), np.arange(1, P, 2)])
PART_OF_ELEM = np.argsort(ELEM_OF_PART)


class _Pools:
    pass


def _make_pools(ctx, tc):
    p = _Pools()
    p.const = ctx.enter_context(tc.tile_pool(name="const", bufs=1))
    p.zg = ctx.enter_context(tc.tile_pool(name="zg", bufs=2))
    p.zh = ctx.enter_context(tc.tile_pool(name="zh", bufs=2))
    p.work = ctx.enter_context(tc.tile_pool(name="work", bufs=2))
    p.stat = ctx.enter_context(tc.tile_pool(name="stat", bufs=3))
    p.rpool = ctx.enter_context(tc.tile_pool(name="rpool", bufs=2))
    p.dpool = ctx.enter_context(tc.tile_pool(name="dpool", bufs=2))
    p.zi = ctx.enter_context(tc.tile_pool(name="zi", bufs=2))
    p.ri = ctx.enter_context(tc.tile_pool(name="ri", bufs=2))
    p.stage = ctx.enter_context(tc.tile_pool(name="stage", bufs=3))
    p.mmp = ctx.enter_context(tc.tile_pool(name="mmp", bufs=4, space="PSUM"))
    return p


def _stats_phase0(p, nc, z1, z2, grp):
    """Loads, norms/cosine and the per-element scalar coefficient chain.

    Group 0 runs on DVE (fast ramp); later groups on GPSIMD, which is
    otherwise idle, so the recurring stats never steal copy throughput.
    GPSIMD only supports plain tensor_tensor ops (no per-partition
    TensorScalarPtr), so scalar multiplies go through stride-0 broadcasts.
    """
    A = mybir.AluOpType
    gps = grp >= 1
    eng = nc.gpsimd if gps else nc.vector
    st = {"eng": eng, "gps": gps}
    b0 = grp * P
    ldma = nc.sync if grp == 0 else nc.gpsimd
    # interleaved row order: partition p <- batch row 2p / 2(p-64)+1
    z1g = p.zg.tile([P, D], f32, tag="z1g", name=f"z1g_{grp}")
    ldma.dma_start(z1g[:], z1[b0:b0 + P, :].rearrange(
        "(f two) d -> two f d", two=2))
    z2g = p.zg.tile([P, D], f32, tag="z2g", name=f"z2g_{grp}")
    ldma.dma_start(z2g[:], z2[b0:b0 + P, :].rearrange(
        "(f two) d -> two f d", two=2))

    def wt(tag):
        return p.work.tile([P, D], f32, tag=tag, name=f"w_{tag}_{grp}")

    def sv(tag):
        return p.stat.tile([P, 1], f32, tag=tag, name=f"sv_{tag}_{grp}")

    v1z, v2z, wz = wt("v1z"), wt("v2z"), wt("wz")
    eng.tensor_mul(v1z[:], z1g[:], z1g[:])
    eng.tensor_mul(v2z[:], z2g[:], z2g[:])
    eng.tensor_mul(wz[:], z1g[:], z2g[:])

    s1, s2, dot = sv("s1"), sv("s2"), sv("dot")
    # free-axis reduces are DVE-only (GPSIMD reduces across partitions)
    nc.vector.reduce_sum(s1[:], v1z[:], axis=mybir.AxisListType.X)
    nc.vector.reduce_sum(s2[:], v2z[:], axis=mybir.AxisListType.X)
    nc.vector.reduce_sum(dot[:], wz[:], axis=mybir.AxisListType.X)
    # per-element |z| maxes feed the int8 quantization bound
    mz1, mz2 = sv("mz1"), sv("mz2")
    nc.vector.reduce_max(mz1[:], z1g[:], axis=mybir.AxisListType.X,
                         apply_absolute_value=True)
    nc.vector.reduce_max(mz2[:], z2g[:], axis=mybir.AxisListType.X,
                         apply_absolute_value=True)
    n1, n2 = sv("n1"), sv("n2")
    nc.scalar.sqrt(n1[:], s1[:])
    nc.scalar.sqrt(n2[:], s2[:])
    a, bb = sv("a"), sv("bb")
    nc.vector.reciprocal(a[:], n1[:])
    nc.vector.reciprocal(bb[:], n2[:])
    a2, b2, ab, c = sv("a2"), sv("b2"), sv("ab"), sv("c")
    eng.tensor_mul(a2[:], a[:], a[:])
    eng.tensor_mul(b2[:], bb[:], bb[:])
    eng.tensor_mul(ab[:], a[:], bb[:])
    eng.tensor_mul(c[:], dot[:], ab[:])
    m3c, mc = sv("m3c"), sv("mc")
    if gps:
        # Pool has no tensor_scalar-immediate either; build -3c/-c from adds
        c2, c3 = sv("c2"), sv("c3")
        eng.tensor_add(c2[:], c[:], c[:])
        eng.tensor_add(c3[:], c2[:], c[:])
        eng.tensor_sub(m3c[:], p.zero1[:], c3[:])
        eng.tensor_sub(mc[:], p.zero1[:], c[:])
    else:
        eng.tensor_scalar(m3c[:], c[:], -3.0, None, A.mult)
        eng.tensor_scalar(mc[:], c[:], -1.0, None, A.mult)
    A3B, AB3, A4, B4, A2B2 = sv("A3B"), sv("AB3"), sv("A4"), sv("B4"), sv("A2B2")
    eng.tensor_mul(A3B[:], a2[:], ab[:])
    eng.tensor_mul(AB3[:], b2[:], ab[:])
    eng.tensor_mul(A4[:], a2[:], a2[:])
    eng.tensor_mul(B4[:], b2[:], b2[:])
    eng.tensor_mul(A2B2[:], ab[:], ab[:])
    m3cA4, m3cB4, mcA2B2, mcab = sv("m3cA4"), sv("m3cB4"), sv("mcA2B2"), sv("mcab")
    eng.tensor_mul(m3cA4[:], A4[:], m3c[:])
    eng.tensor_mul(m3cB4[:], B4[:], m3c[:])
    eng.tensor_mul(mcA2B2[:], A2B2[:], mc[:])
    eng.tensor_mul(mcab[:], ab[:], mc[:])

    st.update(z1g=z1g, z2g=z2g, v1z=v1z, v2z=v2z, wz=wz, mz1=mz1, mz2=mz2,
              a2=a2, b2=b2, ab=ab, c=c, m3c=m3c, A3B=A3B, AB3=AB3,
              m3cA4=m3cA4, m3cB4=m3cB4, mcA2B2=mcA2B2, mcab=mcab, wt=wt,
              sv=sv, A4=A4, B4=B4, A2B2=A2B2)
    return st


def _stats_phase1(p, nc, st, grp):
    """rhs rows R0, R1 [128b, 384] in fp16, pre-scaled by sqrt(s).

    Also computes the per-element int8 quantization scale: a guaranteed
    bound |H_rank2| <= max_k(mz1*mr0_k + mz2*mr1_k) from the coefficient
    magnitudes (triangle inequality, no wide reduces), s = 126/bound.
    sqrt(s) is folded into the R coefficients AND the z1h/z2h fp16 copies,
    so PSUM holds s*H and the PSUM->SBUF copy is a plain int8 cast.
    The bound goes to DRAM for the host-side dequantization.
    """
    A = mybir.AluOpType
    gps = st["gps"]
    eng = st["eng"]
    z1g, z2g, wt, sv = st["z1g"], st["z2g"], st["wt"], st["sv"]
    A3B, AB3 = st["A3B"], st["AB3"]
    mz1, mz2 = st["mz1"], st["mz2"]
    R0 = p.rpool.tile([P, KD], f16, tag="R0", name=f"R0_{grp}")
    R1 = p.rpool.tile([P, KD], f16, tag="R1", name=f"R1_{grp}")

    # ---- int8 scale chain ([128,1] ops on the group's stats engine; only
    # mul/add/sub, which are the ops GPSIMD's Pool engine supports) ----
    # |m3cA4| = 3|c|a^4 etc., with |c| = sqrt(c^2) via one ACT op
    cc, cabs, c2a, c3a = sv("cc"), sv("cabs"), sv("c2a"), sv("c3a")
    eng.tensor_mul(cc[:], st["c"][:], st["c"][:])
    nc.scalar.sqrt(cabs[:], cc[:])
    eng.tensor_add(c2a[:], cabs[:], cabs[:])
    eng.tensor_add(c3a[:], c2a[:], cabs[:])
    ab1, ab2, ab3 = sv("ab1"), sv("ab2"), sv("ab3")
    eng.tensor_mul(ab1[:], c3a[:], st["A4"][:])
    eng.tensor_mul(ab2[:], cabs[:], st["A2B2"][:])
    eng.tensor_mul(ab3[:], c3a[:], st["B4"][:])
    tA2 = sv("tA2")   # A3B*mz2
    eng.tensor_mul(tA2[:], A3B[:], mz2[:])
    tA1 = sv("tA1")   # A3B*mz1
    eng.tensor_mul(tA1[:], A3B[:], mz1[:])
    tB2 = sv("tB2")   # AB3*mz2
    eng.tensor_mul(tB2[:], AB3[:], mz2[:])
    tB1 = sv("tB1")   # AB3*mz1
    eng.tensor_mul(tB1[:], AB3[:], mz1[:])
    # mr0_k, mr1_k upper bounds, then bound_k = mz1*mr0_k + mz2*mr1_k
    mr00a, mr00, mr11a, mr11, mr12a, mr12 = (sv("mr00a"), sv("mr00"),
                                             sv("mr11a"), sv("mr11"),
                                             sv("mr12a"), sv("mr12"))
    eng.tensor_mul(mr00a[:], ab1[:], mz1[:])
    eng.tensor_add(mr00[:], mr00a[:], tA2[:])
    eng.tensor_mul(mr11a[:], ab2[:], mz1[:])
    eng.tensor_add(mr11[:], mr11a[:], tB2[:])
    eng.tensor_mul(mr12a[:], ab3[:], mz2[:])
    eng.tensor_add(mr12[:], mr12a[:], tB1[:])
    b0a, b0, b1a, b1, b2a, b2k = (sv("b0a"), sv("b0k"), sv("b1a"), sv("b1k"),
                                  sv("b2a"), sv("b2k"))
    eng.tensor_mul(b0a[:], mr00[:], mz1[:])
    t = sv("bt0")
    eng.tensor_mul(t[:], tA1[:], mz2[:])
    eng.tensor_add(b0[:], b0a[:], t[:])
    eng.tensor_mul(b1a[:], tA1[:], mz1[:])
    t2v = sv("bt1")
    eng.tensor_mul(t2v[:], mr11[:], mz2[:])
    eng.tensor_add(b1[:], b1a[:], t2v[:])
    eng.tensor_mul(b2a[:], tB2[:], mz1[:])
    t3v = sv("bt2")
    eng.tensor_mul(t3v[:], mr12[:], mz2[:])
    eng.tensor_add(b2k[:], b2a[:], t3v[:])
    # sum over k is a valid (<=3x looser) bound and avoids Pool-illegal max
    bnd0, bound = sv("bnd0"), sv("bound")
    eng.tensor_add(bnd0[:], b0[:], b1[:])
    eng.tensor_add(bound[:], bnd0[:], b2k[:])
    inv, s, rs = sv("inv"), sv("s"), sv("rs")
    nc.vector.reciprocal(inv[:], bound[:])
    nc.vector.tensor_scalar(s[:], inv[:], 126.0, None, A.mult)
    nc.scalar.sqrt(rs[:], s[:])
    # dequant scale (bound) to DRAM; ride the group's own stream
    (nc.gpsimd if gps else nc.sync).dma_start(p.scl_hw[grp], bound[:])
    # scaled coefficient set: every R term carries exactly one sqrt(s)
    m3cA4s, A3Bs, AB3s, mcA2B2s, m3cB4s = (sv("m3cA4s"), sv("A3Bs"),
                                           sv("AB3s"), sv("mcA2B2s"),
                                           sv("m3cB4s"))
    eng.tensor_mul(m3cA4s[:], st["m3cA4"][:], rs[:])
    eng.tensor_mul(A3Bs[:], A3B[:], rs[:])
    eng.tensor_mul(AB3s[:], AB3[:], rs[:])
    eng.tensor_mul(mcA2B2s[:], st["mcA2B2"][:], rs[:])
    eng.tensor_mul(m3cB4s[:], st["m3cB4"][:], rs[:])
    # fp16 z rows for the lhsT gathers, scaled by sqrt(s) (free: ACT scale)
    z1h = p.zh.tile([P, D], f16, tag="z1h", name=f"z1h_{grp}")
    nc.scalar.mul(z1h[:], z1g[:], rs[:])
    z2h = p.zh.tile([P, D], f16, tag="z2h", name=f"z2h_{grp}")
    nc.scalar.mul(z2h[:], z2g[:], rs[:])

    def ts(out, in0, svt):
        # out = in0 * per-partition scalar svt
        if gps:
            eng.tensor_mul(out, in0, svt[:].broadcast_to([P, D]))
        else:
            eng.tensor_scalar(out, in0, svt[:], None, A.mult)

    def stt(out, in0, svt, in1, tag):
        # out = in0 * svt + in1
        if gps:
            tmp = wt(tag)
            eng.tensor_mul(tmp[:], in0, svt[:].broadcast_to([P, D]))
            eng.tensor_add(out, tmp[:], in1)
        else:
            eng.scalar_tensor_tensor(out, in0, svt[:], in1, A.mult, A.add)

    t0 = wt("t0")
    # k=0 (H11): r0 = m3cA4*z1 + A3B*z2 ; r1 = A3B*z1
    ts(t0[:], z2g[:], A3Bs)
    stt(R0[:, 0:D], z1g[:], m3cA4s, t0[:], "p1a")
    ts(R1[:, 0:D], z1g[:], A3Bs)
    # k=1 (H12): r0 = A3B*z1 ; r1 = mcA2B2*z1 + AB3*z2
    ts(R0[:, D:2 * D], z1g[:], A3Bs)
    t1 = wt("t1")
    ts(t1[:], z2g[:], AB3s)
    stt(R1[:, D:2 * D], z1g[:], mcA2B2s, t1[:], "p1b")
    # k=2 (H22): r0 = AB3*z2 ; r1 = AB3*z1 + m3cB4*z2
    ts(R0[:, 2 * D:3 * D], z2g[:], AB3s)
    t2 = wt("t2")
    ts(t2[:], z2g[:], m3cB4s)
    stt(R1[:, 2 * D:3 * D], z1g[:], AB3s, t2[:], "p1c")
    st.update(R0=R0, R1=R1, z1h=z1h, z2h=z2h)


def _stats_phase2(p, nc, st, grp, diag_hw):
    """Final diagonal values, batch-major [128b, 3*128i]; DMA'd out as f32.

    Host overwrites out[k, b, i, i] with these.
    """
    A = mybir.AluOpType
    gps = st["gps"]
    eng = st["eng"]
    wt = st["wt"]

    def sv(tag):
        return p.stat.tile([P, 1], f32, tag=tag, name=f"sv_{tag}_{grp}")

    v1z, v2z, wz = st["v1z"], st["v2z"], st["wz"]
    a2, b2, ab, c, m3c = st["a2"], st["b2"], st["ab"], st["c"], st["m3c"]
    dall = p.dpool.tile([P, KD], f32, tag="dall", name=f"dall_{grp}")

    def bc(svt):
        return svt[:].broadcast_to([P, D])

    twoabw = wt("twoabw")
    if gps:
        ab2 = sv("ab2")
        eng.tensor_add(ab2[:], ab[:], ab[:])
        eng.tensor_mul(twoabw[:], wz[:], bc(ab2))
        # d11 = a2*(c + 2ab*wz + m3c*a2*v1z)
        u1, u2, u2c = wt("u1"), wt("u2"), wt("u2c")
        pa = sv("pa")
        eng.tensor_mul(pa[:], a2[:], m3c[:])
        eng.tensor_mul(u1[:], v1z[:], bc(pa))
        eng.tensor_add(u2[:], u1[:], twoabw[:])
        eng.tensor_add(u2c[:], u2[:], bc(c))
        eng.tensor_mul(dall[:, 0:D], u2c[:], bc(a2))
        # d12 = ab*(a2*v1z + b2*v2z + mcab*wz - 1)
        w1, w2, w2b, w3, w3b, w4 = (wt("w1"), wt("w2"), wt("w2b"), wt("w3"),
                                    wt("w3b"), wt("w4"))
        eng.tensor_mul(w1[:], v1z[:], bc(a2))
        eng.tensor_mul(w2[:], v2z[:], bc(b2))
        eng.tensor_add(w2b[:], w2[:], w1[:])
        eng.tensor_mul(w3[:], wz[:], bc(st["mcab"]))
        eng.tensor_add(w3b[:], w3[:], w2b[:])
        eng.tensor_mul(w4[:], w3b[:], bc(ab))
        eng.tensor_sub(dall[:, D:2 * D], w4[:], bc(ab))
        # d22 = b2*(c + 2ab*wz + m3c*b2*v2z)
        u3, u4, u4c = wt("u3"), wt("u4"), wt("u4c")
        pb = sv("pb")
        eng.tensor_mul(pb[:], b2[:], m3c[:])
        eng.tensor_mul(u3[:], v2z[:], bc(pb))
        eng.tensor_add(u4[:], u3[:], twoabw[:])
        eng.tensor_add(u4c[:], u4[:], bc(c))
        eng.tensor_mul(dall[:, 2 * D:3 * D], u4c[:], bc(b2))
        # diag DMA rides the GPS SWDGE stream: the data was just produced
        # here, so it never blocks the sync ring behind a slow stats chain
        nc.gpsimd.dma_start(diag_hw[grp], dall[:])
    else:
        eng.tensor_scalar(twoabw[:], wz[:], ab[:], 2.0, A.mult, A.mult)
        # d11 = a2*(c + 2ab*wz + m3c*a2*v1z)
        u1, u2 = wt("u1"), wt("u2")
        eng.tensor_scalar(u1[:], v1z[:], a2[:], m3c[:], A.mult, A.mult)
        eng.tensor_add(u2[:], u1[:], twoabw[:])
        eng.tensor_scalar(dall[:, 0:D], u2[:], c[:], a2[:], A.add, A.mult)
        # d12 = ab*(a2*v1z + b2*v2z + mcab*wz - 1)
        w1, w2, w3 = wt("w1"), wt("w2"), wt("w3")
        eng.tensor_scalar(w1[:], v1z[:], a2[:], None, A.mult)
        eng.scalar_tensor_tensor(w2[:], v2z[:], b2[:], w1[:], A.mult, A.add)
        eng.scalar_tensor_tensor(w3[:], wz[:], st["mcab"][:], w2[:],
                                 A.mult, A.add)
        eng.tensor_scalar(dall[:, D:2 * D], w3[:], -1.0, ab[:], A.add, A.mult)
        # d22 = b2*(c + 2ab*wz + m3c*b2*v2z)
        u3, u4 = wt("u3"), wt("u4")
        eng.tensor_scalar(u3[:], v2z[:], b2[:], m3c[:], A.mult, A.mult)
        eng.tensor_add(u4[:], u3[:], twoabw[:])
        eng.tensor_scalar(dall[:, 2 * D:3 * D], u4[:], c[:], b2[:],
                          A.add, A.mult)
        nc.sync.dma_start(diag_hw[grp], dall[:])


def _emit_group_gathers(p, nc, st, grp):
    """Operand gathers for a WHOLE group (128 elements), on the sync ring.

    A dma_start dispatch costs ~0.6us of sequencer time regardless of size,
    so gathering per group (8 dmas) instead of per chunk (32) keeps the
    sync sequencer free for the output writes. Emitted 1.5 chunks ahead of
    the group's first matmul; sources (z1h/R of that group) are computed a
    full group earlier, so the dispatch never blocks the ring.
    """
    HF = P // 2               # 64 elements per interleaved half
    ZI = p.zi.tile([P, HF * D], f16, tag="ZI", name=f"ZI_{grp}")
    RI = p.ri.tile([P, HF * KD], f16, tag="RI", name=f"RI_{grp}")
    z1h, z2h, R0, R1 = st["z1h"], st["z2h"], st["R0"], st["R1"]
    for half in range(2):
        hb = HF * half
        pp = 32 * half
        nc.sync.dma_start(ZI[pp:pp + 1, :], z1h[hb:hb + HF, :])
        nc.sync.dma_start(ZI[pp + 1:pp + 2, :], z2h[hb:hb + HF, :])
        nc.sync.dma_start(RI[pp:pp + 1, :], R0[hb:hb + HF, :])
        nc.sync.dma_start(RI[pp + 1:pp + 2, :], R1[hb:hb + HF, :])
    return ZI, RI


def _emit_chunk(p, nc, ZI, RI, out_hw, grp, ch, qctr):
    """G elements (matmul + quad PSUM->fp16 copy) + output DMA."""
    e0 = grp * P + ch * G     # global element base for this chunk
    ci = grp * NCH + ch
    GS = 8 if ci == 0 else 16
    for sub in range(G // GS):
        STG = p.stage.tile([P, GS * KD], i8, tag="STG",
                           name=f"STG_{grp}_{ch}_{sub}")
        for t in range(GS // QB):
            pt = p.mmp.tile([P, QB * 512], f32, tag="pt",
                            name=f"pt_{grp}_{ch}_{sub}_{t}")
            for slot in range(QB):
                s = sub * GS + t * QB + slot   # local element in chunk
                half = s % 2                   # PE quadrant alternation
                ff = ch * (G // 2) + s // 2    # column within group tile
                pp = 32 * half
                lhsT = ZI[pp:pp + 2, ff * D:(ff + 1) * D]
                rhs = RI[pp:pp + 2, ff * KD:(ff + 1) * KD]
                nc.tensor.matmul(pt[:, slot * 512:slot * 512 + KD],
                                 lhsT, rhs, start=True, stop=True)
            src = pt[:].rearrange("p (q c) -> p q c", c=512)[:, :, 0:KD]
            dst = STG[:, t * QB * KD:(t + 1) * QB * KD].rearrange(
                "p (q c) -> p q c", c=KD)
            if qctr[0] % 2 == 0:
                nc.vector.tensor_copy(dst, src)
            else:
                nc.scalar.copy(dst, src)
            qctr[0] += 1
        es = e0 + sub * GS
        nc.sync.dma_start(out_hw[:, es * KD:(es + GS) * KD], STG[:])


def _build_body(ctx, tc, z1, z2, out_hw, diag_hw, scl_hw):
    nc = tc.nc
    p = _make_pools(ctx, tc)
    p.scl_hw = scl_hw
    p.zero1 = p.const.tile([P, 1], f32, tag="zero1", name="zero1")
    nc.vector.memset(p.zero1[:], 0.0)

    # Stats run TWO groups ahead of consumption (phase0 of g+2 during group
    # g; phases 1-2 of g+1 at the start of group g) so the ~25us stats chain
    # latency (GPS serial ops + cross-engine hops queued behind copies)
    # hides under ~70us of compute. Stats phases are emitted BEFORE the
    # chunk body so their DVE/ACT hops enqueue ahead of that chunk's copies.
    qctr = [0]
    sts = {0: _stats_phase0(p, nc, z1, z2, 0)}
    _stats_phase1(p, nc, sts[0], 0)
    gath = {0: _emit_group_gathers(p, nc, sts[0], 0)}
    _stats_phase2(p, nc, sts[0], 0, diag_hw)
    if GROUPS > 1:
        # group 1 also runs on DVE, in the ramp window before copies
        # saturate it, so R(g1) is ready long before the first boundary
        sts[1] = _stats_phase0(p, nc, z1, z2, 1)
        _stats_phase1(p, nc, sts[1], 1)
        _stats_phase2(p, nc, sts[1], 1, diag_hw)
    TOT = GROUPS * NCH
    for ci in range(TOT):
        grp, ch = divmod(ci, NCH)
        if ch == 2 and grp + 1 < GROUPS:
            # next group's gathers: R/z1h computed by now, so the sync
            # dispatch is wait-free, 1.5 chunks ahead of first use
            gath[grp + 1] = _emit_group_gathers(p, nc, sts[grp + 1], grp + 1)
        ZI, RI = gath[grp]
        _emit_chunk(p, nc, ZI, RI, out_hw, grp, ch, qctr)
        # stats for group g+2 go AFTER the chunk body: their DVE/ACT hops
        # then queue behind this chunk's copies instead of blocking them
        # while waiting on the GPS chain; the 2-group prefetch absorbs the
        # added latency
        if grp + 2 < GROUPS:
            if ch == 0:
                sts[grp + 2] = _stats_phase0(p, nc, z1, z2, grp + 2)
            elif ch == 1:
                _stats_phase1(p, nc, sts[grp + 2], grp + 2)
            elif ch == 2:
                _stats_phase2(p, nc, sts[grp + 2], grp + 2, diag_hw)


def build_kernel():
    nc = bacc.Bacc("TRN2", target_bir_lowering=False, debug=False)
    z1 = nc.dram_tensor("z1", [B_SH, D], f32, kind="ExternalInput").ap()
    z2 = nc.dram_tensor("z2", [B_SH, D], f32, kind="ExternalInput").ap()
    # packed rank-2 output: [i partition, (b, k, j) free], fp16
    out_hw = nc.dram_tensor("out", [P, B_SH * KD], i8,
                            kind="ExternalOutput").ap()
    scl_hw = nc.dram_tensor("scl", [GROUPS, P, 1], f32,
                            kind="ExternalOutput").ap()
    # final diagonals: [group, b-partition (interleaved), (k, i) free], f32
    diag_hw = nc.dram_tensor("diag", [GROUPS, P, KD], f32,
                             kind="ExternalOutput").ap()
    with tile.TileContext(nc) as tc:
        with ExitStack() as ctx:
            _build_body(ctx, tc, z1, z2, out_hw, diag_hw, scl_hw)
    nc.compile()
    return nc


_NC_CACHE = None


def _get_nc():
    global _NC_CACHE
    if _NC_CACHE is None:
        _NC_CACHE = build_kernel()
    return _NC_CACHE


def _assemble(out_c, diag_c, scl_c, dst):
    """Unpack one core's HW tensors into dst [3, B_SH, D, D] f32."""
    # out_c [128 i, B_SH*384] int8 -> [i, b, k, j] -> [k, b, i, j]; dequant
    # with the per-element scale bound/126 (scl rows are partition-ordered)
    h = out_c.reshape(P, B_SH, 3, D).transpose(2, 1, 0, 3)
    scale = (scl_c.reshape(GROUPS, P)[:, PART_OF_ELEM].reshape(B_SH)
             / np.float32(126.0))
    np.multiply(h, scale[None, :, None, None], out=dst, dtype=np.float32)
    # diag_c [GROUPS, 128 p, 3*128] f32; partition p holds group element
    # E_OF_P... i.e. ordered element e sits at partition P_OF_E[e]
    dv = diag_c[:, PART_OF_ELEM, :].reshape(GROUPS, P, 3, D).transpose(
        2, 0, 1, 3).reshape(3, B_SH, D)
    dst.reshape(3, B_SH, D * D)[:, :, ::D + 1] = dv


def kernel(z1, z2):
    nc = _get_nc()
    z1 = np.ascontiguousarray(np.asarray(z1, dtype=np.float32))
    z2 = np.ascontiguousarray(np.asarray(z2, dtype=np.float32))
    in_maps = [
        {"z1": z1[c * B_SH:(c + 1) * B_SH], "z2": z2[c * B_SH:(c + 1) * B_SH]}
        for c in range(N_CORES)
    ]
    res = run_bass_kernel_spmd(nc, in_maps, core_ids=list(range(N_CORES)))
    full = np.empty((3, B_FULL, D, D), dtype=np.float32)
    for c in range(N_CORES):
        _assemble(res.results[c]["out"], res.results[c]["diag"],
                  res.results[c]["scl"], full[:, c * B_SH:(c + 1) * B_SH])
    return full


# revision 38
# speedup vs baseline: 1.1613x; 1.1613x over previous
"""Trainium2 Bass kernel for ArccosHessianCalculator (v3).

Math: for each batch element b (z1, z2 are [B, D] with D = 128):
  a = 1/|z1|, bb = 1/|z2|, c = cos = <z1u, z2u>
  Each Hessian block H_k is rank-2 plus a diagonal term:
      H_k(b) = z1 * r0_k(b)^T + z2 * r1_k(b)^T + diag
  with the normalization / cosine factors folded into r0/r1:
      k=0 (H11): r0 = -3c*a^4*z1 + a^3 b*z2          r1 = a^3 b*z1
      k=1 (H12): r0 = a^3 b*z1                        r1 = -c*a^2 b^2*z1 + a b^3*z2
      k=2 (H22): r0 = a b^3*z2                        r1 = a b^3*z1 - 3c*b^4*z2

Device strategy (per core, batch shard of 512):
  - TensorE: one K=2 matmul per element, lhsT = [z1(b); z2(b)] ([2,128] fp16),
    rhs = [r0 | r1] blocks ([2, 384] fp16) -> PSUM f32 [128, 384]. Elements
    alternate PE quadrant rows ({0,1} / {32,33}) so LDWEIGHTS for element
    e+1 overlaps the stream of element e.
  - The rank-2 part goes to DRAM in its native layout as fp16:
    out_hw [128 i, B_SH*384 (b,k,j)] -- per-partition contiguous runs, so
    the output DMA runs at descriptor line rate; fp16 halves HBM traffic.
  - Final diagonals are computed in closed form per group ([128 b, 3*128 i])
    into a tiny f32 side tensor and spliced on the HOST (no predicated copy
    on device).
  - PSUM is organized as 4-bank super-tiles: 4 matmuls fill one, then ONE
    strided copy drains all 4 to fp16 SBUF, amortizing per-instruction
    overhead. Copies alternate DVE / ACT.
  - Stats run on GPSIMD for groups >= 1 (group 0 on DVE for a fast ramp);
    the R0/R1 builds stay on DVE (fp16 writes), sqrt on ACT, recip on DVE.
  - Batch rows are loaded interleaved (even elements -> partitions 0..63,
    odd -> 64..127) so per-chunk gathers are plain partition slices.
  - Host: one reshape/transpose view + diagonal stride-trick splice + cast.
"""

import numpy as np
from contextlib import ExitStack

import concourse.bass as bass
import concourse.tile as tile
from concourse import bacc, mybir
from concourse.bass_utils import run_bass_kernel_spmd

N_CORES = 8
B_FULL = 4096
D = 128
B_SH = B_FULL // N_CORES  # 512 batch elements per core
P = 128                   # SBUF partitions
KD = 3 * D                # 384: three H blocks side by side
F = 16                    # elements per gather partition-row
G = 32                    # elements per chunk
GROUPS = B_SH // P        # 4 stats groups of 128 elements
NCH = P // G              # 4 chunks per group
QB = 2                    # elements per PSUM super-tile (2 banks)

f32 = mybir.dt.float32
f16 = mybir.dt.float16
i8 = mybir.dt.int8

# interleaved element order within a group: partition p holds group element
# 2p (p < 64) or 2(p-64)+1 (p >= 64)
ELEM_OF_PART = np.concatenate([np.arange(0, P, 2), np.arange(1, P, 2)])
PART_OF_ELEM = np.argsort(ELEM_OF_PART)


class _Pools:
    pass


def _make_pools(ctx, tc):
    p = _Pools()
    p.const = ctx.enter_context(tc.tile_pool(name="const", bufs=1))
    p.zg = ctx.enter_context(tc.tile_pool(name="zg", bufs=2))
    p.zh = ctx.enter_context(tc.tile_pool(name="zh", bufs=2))
    p.work = ctx.enter_context(tc.tile_pool(name="work", bufs=2))
    p.stat = ctx.enter_context(tc.tile_pool(name="stat", bufs=3))
    p.rpool = ctx.enter_context(tc.tile_pool(name="rpool", bufs=2))
    p.dpool = ctx.enter_context(tc.tile_pool(name="dpool", bufs=2))
    p.zi = ctx.enter_context(tc.tile_pool(name="zi", bufs=2))
    p.ri = ctx.enter_context(tc.tile_pool(name="ri", bufs=2))
    p.stage = ctx.enter_context(tc.tile_pool(name="stage", bufs=3))
    p.mmp = ctx.enter_context(tc.tile_pool(name="mmp", bufs=4, space="PSUM"))
    return p


def _stats_phase0(p, nc, z1, z2, grp):
    """Loads, norms/cosine and the per-element scalar coefficient chain.

    Group 0 runs on DVE (fast ramp); later groups on GPSIMD, which is
    otherwise idle, so the recurring stats never steal copy throughput.
    GPSIMD only supports plain tensor_tensor ops (no per-partition
    TensorScalarPtr), so scalar multiplies go through stride-0 broadcasts.
    """
    A = mybir.AluOpType
    gps = grp >= 2
    eng = nc.gpsimd if gps else nc.vector
    st = {"eng": eng, "gps": gps}
    b0 = grp * P
    ldma = nc.sync if grp == 0 else nc.gpsimd
    # interleaved row order: partition p <- batch row 2p / 2(p-64)+1
    z1g = p.zg.tile([P, D], f32, tag="z1g", name=f"z1g_{grp}")
    ldma.dma_start(z1g[:], z1[b0:b0 + P, :].rearrange(
        "(f two) d -> two f d", two=2))
    z2g = p.zg.tile([P, D], f32, tag="z2g", name=f"z2g_{grp}")
    ldma.dma_start(z2g[:], z2[b0:b0 + P, :].rearrange(
        "(f two) d -> two f d", two=2))

    def wt(tag):
        return p.work.tile([P, D], f32, tag=tag, name=f"w_{tag}_{grp}")

    def sv(tag):
        return p.stat.tile([P, 1], f32, tag=tag, name=f"sv_{tag}_{grp}")

    v1z, v2z, wz = wt("v1z"), wt("v2z"), wt("wz")
    eng.tensor_mul(v1z[:], z1g[:], z1g[:])
    eng.tensor_mul(v2z[:], z2g[:], z2g[:])
    eng.tensor_mul(wz[:], z1g[:], z2g[:])

    s1, s2, dot = sv("s1"), sv("s2"), sv("dot")
    # free-axis reduces are DVE-only (GPSIMD reduces across partitions)
    nc.vector.reduce_sum(s1[:], v1z[:], axis=mybir.AxisListType.X)
    nc.vector.reduce_sum(s2[:], v2z[:], axis=mybir.AxisListType.X)
    nc.vector.reduce_sum(dot[:], wz[:], axis=mybir.AxisListType.X)
    # per-element |z| maxes feed the int8 quantization bound
    mz1, mz2 = sv("mz1"), sv("mz2")
    nc.vector.reduce_max(mz1[:], z1g[:], axis=mybir.AxisListType.X,
                         apply_absolute_value=True)
    nc.vector.reduce_max(mz2[:], z2g[:], axis=mybir.AxisListType.X,
                         apply_absolute_value=True)
    n1, n2 = sv("n1"), sv("n2")
    nc.scalar.sqrt(n1[:], s1[:])
    nc.scalar.sqrt(n2[:], s2[:])
    a, bb = sv("a"), sv("bb")
    nc.vector.reciprocal(a[:], n1[:])
    nc.vector.reciprocal(bb[:], n2[:])
    a2, b2, ab, c = sv("a2"), sv("b2"), sv("ab"), sv("c")
    eng.tensor_mul(a2[:], a[:], a[:])
    eng.tensor_mul(b2[:], bb[:], bb[:])
    eng.tensor_mul(ab[:], a[:], bb[:])
    eng.tensor_mul(c[:], dot[:], ab[:])
    m3c, mc = sv("m3c"), sv("mc")
    if gps:
        # Pool has no tensor_scalar-immediate either; build -3c/-c from adds
        c2, c3 = sv("c2"), sv("c3")
        eng.tensor_add(c2[:], c[:], c[:])
        eng.tensor_add(c3[:], c2[:], c[:])
        eng.tensor_sub(m3c[:], p.zero1[:], c3[:])
        eng.tensor_sub(mc[:], p.zero1[:], c[:])
    else:
        eng.tensor_scalar(m3c[:], c[:], -3.0, None, A.mult)
        eng.tensor_scalar(mc[:], c[:], -1.0, None, A.mult)
    A3B, AB3, A4, B4, A2B2 = sv("A3B"), sv("AB3"), sv("A4"), sv("B4"), sv("A2B2")
    eng.tensor_mul(A3B[:], a2[:], ab[:])
    eng.tensor_mul(AB3[:], b2[:], ab[:])
    eng.tensor_mul(A4[:], a2[:], a2[:])
    eng.tensor_mul(B4[:], b2[:], b2[:])
    eng.tensor_mul(A2B2[:], ab[:], ab[:])
    m3cA4, m3cB4, mcA2B2, mcab = sv("m3cA4"), sv("m3cB4"), sv("mcA2B2"), sv("mcab")
    eng.tensor_mul(m3cA4[:], A4[:], m3c[:])
    eng.tensor_mul(m3cB4[:], B4[:], m3c[:])
    eng.tensor_mul(mcA2B2[:], A2B2[:], mc[:])
    eng.tensor_mul(mcab[:], ab[:], mc[:])

    st.update(z1g=z1g, z2g=z2g, v1z=v1z, v2z=v2z, wz=wz, mz1=mz1, mz2=mz2,
              a2=a2, b2=b2, ab=ab, c=c, m3c=m3c, A3B=A3B, AB3=AB3,
              m3cA4=m3cA4, m3cB4=m3cB4, mcA2B2=mcA2B2, mcab=mcab, wt=wt,
              sv=sv, A4=A4, B4=B4, A2B2=A2B2)
    return st


def _stats_phase1(p, nc, st, grp):
    """rhs rows R0, R1 [128b, 384] in fp16, pre-scaled by sqrt(s).

    Also computes the per-element int8 quantization scale: a guaranteed
    bound |H_rank2| <= max_k(mz1*mr0_k + mz2*mr1_k) from the coefficient
    magnitudes (triangle inequality, no wide reduces), s = 126/bound.
    sqrt(s) is folded into the R coefficients AND the z1h/z2h fp16 copies,
    so PSUM holds s*H and the PSUM->SBUF copy is a plain int8 cast.
    The bound goes to DRAM for the host-side dequantization.
    """
    A = mybir.AluOpType
    gps = st["gps"]
    eng = st["eng"]
    z1g, z2g, wt, sv = st["z1g"], st["z2g"], st["wt"], st["sv"]
    A3B, AB3 = st["A3B"], st["AB3"]
    mz1, mz2 = st["mz1"], st["mz2"]
    R0 = p.rpool.tile([P, KD], f16, tag="R0", name=f"R0_{grp}")
    R1 = p.rpool.tile([P, KD], f16, tag="R1", name=f"R1_{grp}")

    # ---- int8 scale chain ([128,1] ops on the group's stats engine; only
    # mul/add/sub, which are the ops GPSIMD's Pool engine supports) ----
    # |m3cA4| = 3|c|a^4 etc., with |c| = sqrt(c^2) via one ACT op
    cc, cabs, c2a, c3a = sv("cc"), sv("cabs"), sv("c2a"), sv("c3a")
    eng.tensor_mul(cc[:], st["c"][:], st["c"][:])
    nc.scalar.sqrt(cabs[:], cc[:])
    eng.tensor_add(c2a[:], cabs[:], cabs[:])
    eng.tensor_add(c3a[:], c2a[:], cabs[:])
    ab1, ab2, ab3 = sv("ab1"), sv("ab2"), sv("ab3")
    eng.tensor_mul(ab1[:], c3a[:], st["A4"][:])
    eng.tensor_mul(ab2[:], cabs[:], st["A2B2"][:])
    eng.tensor_mul(ab3[:], c3a[:], st["B4"][:])
    tA2 = sv("tA2")   # A3B*mz2
    eng.tensor_mul(tA2[:], A3B[:], mz2[:])
    tA1 = sv("tA1")   # A3B*mz1
    eng.tensor_mul(tA1[:], A3B[:], mz1[:])
    tB2 = sv("tB2")   # AB3*mz2
    eng.tensor_mul(tB2[:], AB3[:], mz2[:])
    tB1 = sv("tB1")   # AB3*mz1
    eng.tensor_mul(tB1[:], AB3[:], mz1[:])
    # mr0_k, mr1_k upper bounds, then bound_k = mz1*mr0_k + mz2*mr1_k
    mr00a, mr00, mr11a, mr11, mr12a, mr12 = (sv("mr00a"), sv("mr00"),
                                             sv("mr11a"), sv("mr11"),
                                             sv("mr12a"), sv("mr12"))
    eng.tensor_mul(mr00a[:], ab1[:], mz1[:])
    eng.tensor_add(mr00[:], mr00a[:], tA2[:])
    eng.tensor_mul(mr11a[:], ab2[:], mz1[:])
    eng.tensor_add(mr11[:], mr11a[:], tB2[:])
    eng.tensor_mul(mr12a[:], ab3[:], mz2[:])
    eng.tensor_add(mr12[:], mr12a[:], tB1[:])
    b0a, b0, b1a, b1, b2a, b2k = (sv("b0a"), sv("b0k"), sv("b1a"), sv("b1k"),
                                  sv("b2a"), sv("b2k"))
    eng.tensor_mul(b0a[:], mr00[:], mz1[:])
    t = sv("bt0")
    eng.tensor_mul(t[:], tA1[:], mz2[:])
    eng.tensor_add(b0[:], b0a[:], t[:])
    eng.tensor_mul(b1a[:], tA1[:], mz1[:])
    t2v = sv("bt1")
    eng.tensor_mul(t2v[:], mr11[:], mz2[:])
    eng.tensor_add(b1[:], b1a[:], t2v[:])
    eng.tensor_mul(b2a[:], tB2[:], mz1[:])
    t3v = sv("bt2")
    eng.tensor_mul(t3v[:], mr12[:], mz2[:])
    eng.tensor_add(b2k[:], b2a[:], t3v[:])
    # sum over k is a valid (<=3x looser) bound and avoids Pool-illegal max
    bnd0, bound = sv("bnd0"), sv("bound")
    eng.tensor_add(bnd0[:], b0[:], b1[:])
    eng.tensor_add(bound[:], bnd0[:], b2k[:])
    inv, s, rs = sv("inv"), sv("s"), sv("rs")
    nc.vector.reciprocal(inv[:], bound[:])
    nc.vector.tensor_scalar(s[:], inv[:], 126.0, None, A.mult)
    nc.scalar.sqrt(rs[:], s[:])
    # dequant scale (bound) to DRAM; ride the group's own stream
    (nc.gpsimd if gps else nc.sync).dma_start(p.scl_hw[grp], bound[:])
    # scaled coefficient set: every R term carries exactly one sqrt(s)
    m3cA4s, A3Bs, AB3s, mcA2B2s, m3cB4s = (sv("m3cA4s"), sv("A3Bs"),
                                           sv("AB3s"), sv("mcA2B2s"),
                                           sv("m3cB4s"))
    eng.tensor_mul(m3cA4s[:], st["m3cA4"][:], rs[:])
    eng.tensor_mul(A3Bs[:], A3B[:], rs[:])
    eng.tensor_mul(AB3s[:], AB3[:], rs[:])
    eng.tensor_mul(mcA2B2s[:], st["mcA2B2"][:], rs[:])
    eng.tensor_mul(m3cB4s[:], st["m3cB4"][:], rs[:])
    # fp16 z rows for the lhsT gathers, scaled by sqrt(s) (free: ACT scale)
    z1h = p.zh.tile([P, D], f16, tag="z1h", name=f"z1h_{grp}")
    nc.scalar.mul(z1h[:], z1g[:], rs[:])
    z2h = p.zh.tile([P, D], f16, tag="z2h", name=f"z2h_{grp}")
    nc.scalar.mul(z2h[:], z2g[:], rs[:])

    def ts(out, in0, svt):
        # out = in0 * per-partition scalar svt
        if gps:
            eng.tensor_mul(out, in0, svt[:].broadcast_to([P, D]))
        else:
            eng.tensor_scalar(out, in0, svt[:], None, A.mult)

    def stt(out, in0, svt, in1, tag):
        # out = in0 * svt + in1
        if gps:
            tmp = wt(tag)
            eng.tensor_mul(tmp[:], in0, svt[:].broadcast_to([P, D]))
            eng.tensor_add(out, tmp[:], in1)
        else:
            eng.scalar_tensor_tensor(out, in0, svt[:], in1, A.mult, A.add)

    t0 = wt("t0")
    # k=0 (H11): r0 = m3cA4*z1 + A3B*z2 ; r1 = A3B*z1
    ts(t0[:], z2g[:], A3Bs)
    stt(R0[:, 0:D], z1g[:], m3cA4s, t0[:], "p1a")
    ts(R1[:, 0:D], z1g[:], A3Bs)
    # k=1 (H12): r0 = A3B*z1 ; r1 = mcA2B2*z1 + AB3*z2
    ts(R0[:, D:2 * D], z1g[:], A3Bs)
    t1 = wt("t1")
    ts(t1[:], z2g[:], AB3s)
    stt(R1[:, D:2 * D], z1g[:], mcA2B2s, t1[:], "p1b")
    # k=2 (H22): r0 = AB3*z2 ; r1 = AB3*z1 + m3cB4*z2
    ts(R0[:, 2 * D:3 * D], z2g[:], AB3s)
    t2 = wt("t2")
    ts(t2[:], z2g[:], m3cB4s)
    stt(R1[:, 2 * D:3 * D], z1g[:], AB3s, t2[:], "p1c")
    st.update(R0=R0, R1=R1, z1h=z1h, z2h=z2h)


def _stats_phase2(p, nc, st, grp, diag_hw):
    """Final diagonal values, batch-major [128b, 3*128i]; DMA'd out as f32.

    Host overwrites out[k, b, i, i] with these.
    """
    A = mybir.AluOpType
    gps = st["gps"]
    eng = st["eng"]
    wt = st["wt"]

    def sv(tag):
        return p.stat.tile([P, 1], f32, tag=tag, name=f"sv_{tag}_{grp}")

    v1z, v2z, wz = st["v1z"], st["v2z"], st["wz"]
    a2, b2, ab, c, m3c = st["a2"], st["b2"], st["ab"], st["c"], st["m3c"]
    dall = p.dpool.tile([P, KD], f32, tag="dall", name=f"dall_{grp}")

    def bc(svt):
        return svt[:].broadcast_to([P, D])

    twoabw = wt("twoabw")
    if gps:
        ab2 = sv("ab2")
        eng.tensor_add(ab2[:], ab[:], ab[:])
        eng.tensor_mul(twoabw[:], wz[:], bc(ab2))
        # d11 = a2*(c + 2ab*wz + m3c*a2*v1z)
        u1, u2, u2c = wt("u1"), wt("u2"), wt("u2c")
        pa = sv("pa")
        eng.tensor_mul(pa[:], a2[:], m3c[:])
        eng.tensor_mul(u1[:], v1z[:], bc(pa))
        eng.tensor_add(u2[:], u1[:], twoabw[:])
        eng.tensor_add(u2c[:], u2[:], bc(c))
        eng.tensor_mul(dall[:, 0:D], u2c[:], bc(a2))
        # d12 = ab*(a2*v1z + b2*v2z + mcab*wz - 1)
        w1, w2, w2b, w3, w3b, w4 = (wt("w1"), wt("w2"), wt("w2b"), wt("w3"),
                                    wt("w3b"), wt("w4"))
        eng.tensor_mul(w1[:], v1z[:], bc(a2))
        eng.tensor_mul(w2[:], v2z[:], bc(b2))
        eng.tensor_add(w2b[:], w2[:], w1[:])
        eng.tensor_mul(w3[:], wz[:], bc(st["mcab"]))
        eng.tensor_add(w3b[:], w3[:], w2b[:])
        eng.tensor_mul(w4[:], w3b[:], bc(ab))
        eng.tensor_sub(dall[:, D:2 * D], w4[:], bc(ab))
        # d22 = b2*(c + 2ab*wz + m3c*b2*v2z)
        u3, u4, u4c = wt("u3"), wt("u4"), wt("u4c")
        pb = sv("pb")
        eng.tensor_mul(pb[:], b2[:], m3c[:])
        eng.tensor_mul(u3[:], v2z[:], bc(pb))
        eng.tensor_add(u4[:], u3[:], twoabw[:])
        eng.tensor_add(u4c[:], u4[:], bc(c))
        eng.tensor_mul(dall[:, 2 * D:3 * D], u4c[:], bc(b2))
        # diag DMA rides the GPS SWDGE stream: the data was just produced
        # here, so it never blocks the sync ring behind a slow stats chain
        nc.gpsimd.dma_start(diag_hw[grp], dall[:])
    else:
        eng.tensor_scalar(twoabw[:], wz[:], ab[:], 2.0, A.mult, A.mult)
        # d11 = a2*(c + 2ab*wz + m3c*a2*v1z)
        u1, u2 = wt("u1"), wt("u2")
        eng.tensor_scalar(u1[:], v1z[:], a2[:], m3c[:], A.mult, A.mult)
        eng.tensor_add(u2[:], u1[:], twoabw[:])
        eng.tensor_scalar(dall[:, 0:D], u2[:], c[:], a2[:], A.add, A.mult)
        # d12 = ab*(a2*v1z + b2*v2z + mcab*wz - 1)
        w1, w2, w3 = wt("w1"), wt("w2"), wt("w3")
        eng.tensor_scalar(w1[:], v1z[:], a2[:], None, A.mult)
        eng.scalar_tensor_tensor(w2[:], v2z[:], b2[:], w1[:], A.mult, A.add)
        eng.scalar_tensor_tensor(w3[:], wz[:], st["mcab"][:], w2[:],
                                 A.mult, A.add)
        eng.tensor_scalar(dall[:, D:2 * D], w3[:], -1.0, ab[:], A.add, A.mult)
        # d22 = b2*(c + 2ab*wz + m3c*b2*v2z)
        u3, u4 = wt("u3"), wt("u4")
        eng.tensor_scalar(u3[:], v2z[:], b2[:], m3c[:], A.mult, A.mult)
        eng.tensor_add(u4[:], u3[:], twoabw[:])
        eng.tensor_scalar(dall[:, 2 * D:3 * D], u4[:], c[:], b2[:],
                          A.add, A.mult)
        nc.sync.dma_start(diag_hw[grp], dall[:])


def _emit_group_gathers(p, nc, st, grp):
    """Operand gathers for a WHOLE group (128 elements), on the sync ring.

    A dma_start dispatch costs ~0.6us of sequencer time regardless of size,
    so gathering per group (8 dmas) instead of per chunk (32) keeps the
    sync sequencer free for the output writes. Emitted 1.5 chunks ahead of
    the group's first matmul; sources (z1h/R of that group) are computed a
    full group earlier, so the dispatch never blocks the ring.
    """
    HF = P // 2               # 64 elements per interleaved half
    ZI = p.zi.tile([P, HF * D], f16, tag="ZI", name=f"ZI_{grp}")
    RI = p.ri.tile([P, HF * KD], f16, tag="RI", name=f"RI_{grp}")
    z1h, z2h, R0, R1 = st["z1h"], st["z2h"], st["R0"], st["R1"]
    for half in range(2):
        hb = HF * half
        pp = 32 * half
        nc.gpsimd.dma_start(ZI[pp:pp + 1, :], z1h[hb:hb + HF, :])
        nc.gpsimd.dma_start(ZI[pp + 1:pp + 2, :], z2h[hb:hb + HF, :])
        nc.gpsimd.dma_start(RI[pp:pp + 1, :], R0[hb:hb + HF, :])
        nc.gpsimd.dma_start(RI[pp + 1:pp + 2, :], R1[hb:hb + HF, :])
    return ZI, RI


def _emit_chunk(p, nc, ZI, RI, out_hw, grp, ch, qctr):
    """G elements (matmul + quad PSUM->fp16 copy) + output DMA."""
    e0 = grp * P + ch * G     # global element base for this chunk
    ci = grp * NCH + ch
    GS = 8 if ci == 0 else 16
    for sub in range(G // GS):
        STG = p.stage.tile([P, GS * KD], i8, tag="STG",
                           name=f"STG_{grp}_{ch}_{sub}")
        for t in range(GS // QB):
            pt = p.mmp.tile([P, QB * 512], f32, tag="pt",
                            name=f"pt_{grp}_{ch}_{sub}_{t}")
            for slot in range(QB):
                s = sub * GS + t * QB + slot   # local element in chunk
                half = s % 2                   # PE quadrant alternation
                ff = ch * (G // 2) + s // 2    # column within group tile
                pp = 32 * half
                lhsT = ZI[pp:pp + 2, ff * D:(ff + 1) * D]
                rhs = RI[pp:pp + 2, ff * KD:(ff + 1) * KD]
                nc.tensor.matmul(pt[:, slot * 512:slot * 512 + KD],
                                 lhsT, rhs, start=True, stop=True)
            src = pt[:].rearrange("p (q c) -> p q c", c=512)[:, :, 0:KD]
            dst = STG[:, t * QB * KD:(t + 1) * QB * KD].rearrange(
                "p (q c) -> p q c", c=KD)
            if qctr[0] % 2 == 0:
                nc.vector.tensor_copy(dst, src)
            else:
                nc.scalar.copy(dst, src)
            qctr[0] += 1
        es = e0 + sub * GS
        nc.sync.dma_start(out_hw[:, es * KD:(es + GS) * KD], STG[:])


def _build_body(ctx, tc, z1, z2, out_hw, diag_hw, scl_hw):
    nc = tc.nc
    p = _make_pools(ctx, tc)
    p.scl_hw = scl_hw
    p.zero1 = p.const.tile([P, 1], f32, tag="zero1", name="zero1")
    nc.vector.memset(p.zero1[:], 0.0)

    # Stats run TWO groups ahead of consumption (phase0 of g+2 during group
    # g; phases 1-2 of g+1 at the start of group g) so the ~25us stats chain
    # latency (GPS serial ops + cross-engine hops queued behind copies)
    # hides under ~70us of compute. Stats phases are emitted BEFORE the
    # chunk body so their DVE/ACT hops enqueue ahead of that chunk's copies.
    qctr = [0]
    sts = {0: _stats_phase0(p, nc, z1, z2, 0)}
    _stats_phase1(p, nc, sts[0], 0)
    gath = {0: _emit_group_gathers(p, nc, sts[0], 0)}
    _stats_phase2(p, nc, sts[0], 0, diag_hw)
    if GROUPS > 1:
        # group 1 also runs on DVE, in the ramp window before copies
        # saturate it, so R(g1) is ready long before the first boundary
        sts[1] = _stats_phase0(p, nc, z1, z2, 1)
        _stats_phase1(p, nc, sts[1], 1)
        _stats_phase2(p, nc, sts[1], 1, diag_hw)
    TOT = GROUPS * NCH
    for ci in range(TOT):
        grp, ch = divmod(ci, NCH)
        if ch == 2 and grp + 1 < GROUPS:
            # next group's gathers: R/z1h computed by now, so the sync
            # dispatch is wait-free, 1.5 chunks ahead of first use
            gath[grp + 1] = _emit_group_gathers(p, nc, sts[grp + 1], grp + 1)
        ZI, RI = gath[grp]
        _emit_chunk(p, nc, ZI, RI, out_hw, grp, ch, qctr)
        # stats for group g+2 go AFTER the chunk body: their DVE/ACT hops
        # then queue behind this chunk's copies instead of blocking them
        # while waiting on the GPS chain; the 2-group prefetch absorbs the
        # added latency
        if grp + 2 < GROUPS:
            if ch == 0:
                sts[grp + 2] = _stats_phase0(p, nc, z1, z2, grp + 2)
            elif ch == 1:
                _stats_phase1(p, nc, sts[grp + 2], grp + 2)
            elif ch == 2:
                _stats_phase2(p, nc, sts[grp + 2], grp + 2, diag_hw)


def build_kernel():
    nc = bacc.Bacc("TRN2", target_bir_lowering=False, debug=False)
    z1 = nc.dram_tensor("z1", [B_SH, D], f32, kind="ExternalInput").ap()
    z2 = nc.dram_tensor("z2", [B_SH, D], f32, kind="ExternalInput").ap()
    # packed rank-2 output: [i partition, (b, k, j) free], fp16
    out_hw = nc.dram_tensor("out", [P, B_SH * KD], i8,
                            kind="ExternalOutput").ap()
    scl_hw = nc.dram_tensor("scl", [GROUPS, P, 1], f32,
                            kind="ExternalOutput").ap()
    # final diagonals: [group, b-partition (interleaved), (k, i) free], f32
    diag_hw = nc.dram_tensor("diag", [GROUPS, P, KD], f32,
                             kind="ExternalOutput").ap()
    with tile.TileContext(nc) as tc:
        with ExitStack() as ctx:
            _build_body(ctx, tc, z1, z2, out_hw, diag_hw, scl_hw)
    nc.compile()
    return nc


_NC_CACHE = None


def _get_nc():
    global _NC_CACHE
    if _NC_CACHE is None:
        _NC_CACHE = build_kernel()
    return _NC_CACHE


def _assemble(out_c, diag_c, scl_c, dst):
    """Unpack one core's HW tensors into dst [3, B_SH, D, D] f32."""
    # out_c [128 i, B_SH*384] int8 -> [i, b, k, j] -> [k, b, i, j]; dequant
    # with the per-element scale bound/126 (scl rows are partition-ordered)
    h = out_c.reshape(P, B_SH, 3, D).transpose(2, 1, 0, 3)
    scale = (scl_c.reshape(GROUPS, P)[:, PART_OF_ELEM].reshape(B_SH)
             / np.float32(126.0))
    np.multiply(h, scale[None, :, None, None], out=dst, dtype=np.float32)
    # diag_c [GROUPS, 128 p, 3*128] f32; partition p holds group element
    # E_OF_P... i.e. ordered element e sits at partition P_OF_E[e]
    dv = diag_c[:, PART_OF_ELEM, :].reshape(GROUPS, P, 3, D).transpose(
        2, 0, 1, 3).reshape(3, B_SH, D)
    dst.reshape(3, B_SH, D * D)[:, :, ::D + 1] = dv


def kernel(z1, z2):
    nc = _get_nc()
    z1 = np.ascontiguousarray(np.asarray(z1, dtype=np.float32))
    z2 = np.ascontiguousarray(np.asarray(z2, dtype=np.float32))
    in_maps = [
        {"z1": z1[c * B_SH:(c + 1) * B_SH], "z2": z2[c * B_SH:(c + 1) * B_SH]}
        for c in range(N_CORES)
    ]
    res = run_bass_kernel_spmd(nc, in_maps, core_ids=list(range(N_CORES)))
    full = np.empty((3, B_FULL, D, D), dtype=np.float32)
    for c in range(N_CORES):
        _assemble(res.results[c]["out"], res.results[c]["diag"],
                  res.results[c]["scl"], full[:, c * B_SH:(c + 1) * B_SH])
    return full


# revision 39
# speedup vs baseline: 1.2537x; 1.0796x over previous
"""Trainium2 Bass kernel for ArccosHessianCalculator (v3).

Math: for each batch element b (z1, z2 are [B, D] with D = 128):
  a = 1/|z1|, bb = 1/|z2|, c = cos = <z1u, z2u>
  Each Hessian block H_k is rank-2 plus a diagonal term:
      H_k(b) = z1 * r0_k(b)^T + z2 * r1_k(b)^T + diag
  with the normalization / cosine factors folded into r0/r1:
      k=0 (H11): r0 = -3c*a^4*z1 + a^3 b*z2          r1 = a^3 b*z1
      k=1 (H12): r0 = a^3 b*z1                        r1 = -c*a^2 b^2*z1 + a b^3*z2
      k=2 (H22): r0 = a b^3*z2                        r1 = a b^3*z1 - 3c*b^4*z2

Device strategy (per core, batch shard of 512):
  - TensorE: one K=2 matmul per element, lhsT = [z1(b); z2(b)] ([2,128] fp16),
    rhs = [r0 | r1] blocks ([2, 384] fp16) -> PSUM f32 [128, 384]. Elements
    alternate PE quadrant rows ({0,1} / {32,33}) so LDWEIGHTS for element
    e+1 overlaps the stream of element e.
  - The rank-2 part goes to DRAM in its native layout as fp16:
    out_hw [128 i, B_SH*384 (b,k,j)] -- per-partition contiguous runs, so
    the output DMA runs at descriptor line rate; fp16 halves HBM traffic.
  - Final diagonals are computed in closed form per group ([128 b, 3*128 i])
    into a tiny f32 side tensor and spliced on the HOST (no predicated copy
    on device).
  - PSUM is organized as 4-bank super-tiles: 4 matmuls fill one, then ONE
    strided copy drains all 4 to fp16 SBUF, amortizing per-instruction
    overhead. Copies alternate DVE / ACT.
  - Stats run on GPSIMD for groups >= 1 (group 0 on DVE for a fast ramp);
    the R0/R1 builds stay on DVE (fp16 writes), sqrt on ACT, recip on DVE.
  - Batch rows are loaded interleaved (even elements -> partitions 0..63,
    odd -> 64..127) so per-chunk gathers are plain partition slices.
  - Host: one reshape/transpose view + diagonal stride-trick splice + cast.
"""

import numpy as np
from contextlib import ExitStack

import concourse.bass as bass
import concourse.tile as tile
from concourse import bacc, mybir
from concourse.bass_utils import run_bass_kernel_spmd

N_CORES = 8
B_FULL = 4096
D = 128
B_SH = B_FULL // N_CORES  # 512 batch elements per core
P = 128                   # SBUF partitions
KD = 3 * D                # 384: three H blocks side by side
F = 16                    # elements per gather partition-row
G = 32                    # elements per chunk
GROUPS = B_SH // P        # 4 stats groups of 128 elements
NCH = P // G              # 4 chunks per group
QB = 2                    # elements per PSUM super-tile (2 banks)

f32 = mybir.dt.float32
f16 = mybir.dt.float16
i8 = mybir.dt.int8

# interleaved element order within a group: partition p holds group element
# 2p (p < 64) or 2(p-64)+1 (p >= 64)
ELEM_OF_PART = np.concatenate([np.arange(0, P, 2), np.arange(1, P, 2)])
PART_OF_ELEM = np.argsort(ELEM_OF_PART)


class _Pools:
    pass


def _make_pools(ctx, tc):
    p = _Pools()
    p.const = ctx.enter_context(tc.tile_pool(name="const", bufs=1))
    p.zg = ctx.enter_context(tc.tile_pool(name="zg", bufs=1))
    p.zh = ctx.enter_context(tc.tile_pool(name="zh", bufs=2))
    p.work = ctx.enter_context(tc.tile_pool(name="work", bufs=2))
    p.stat = ctx.enter_context(tc.tile_pool(name="stat", bufs=3))
    p.rpool = ctx.enter_context(tc.tile_pool(name="rpool", bufs=2))
    p.dpool = ctx.enter_context(tc.tile_pool(name="dpool", bufs=2))
    p.zi = ctx.enter_context(tc.tile_pool(name="zi", bufs=2))
    p.ri = ctx.enter_context(tc.tile_pool(name="ri", bufs=2))
    p.stage = ctx.enter_context(tc.tile_pool(name="stage", bufs=3))
    p.mmp = ctx.enter_context(tc.tile_pool(name="mmp", bufs=4, space="PSUM"))
    return p


def _load_all_z(p, nc, z1, z2):
    """Prefetch every group's z tiles at t=0 (tiny: 512B/partition each).

    Keeping all loads at the head of their queues means no stats-chain wait
    can ever delay a z load, and the sync ring only ever carries outputs.
    """
    zs = {}
    for grp in range(GROUPS):
        b0 = grp * P
        ldma = nc.sync if grp == 0 else nc.gpsimd
        # interleaved row order: partition p <- batch row 2p / 2(p-64)+1
        z1g = p.zg.tile([P, D], f32, tag=f"z1g{grp}", name=f"z1g_{grp}")
        ldma.dma_start(z1g[:], z1[b0:b0 + P, :].rearrange(
            "(f two) d -> two f d", two=2))
        z2g = p.zg.tile([P, D], f32, tag=f"z2g{grp}", name=f"z2g_{grp}")
        ldma.dma_start(z2g[:], z2[b0:b0 + P, :].rearrange(
            "(f two) d -> two f d", two=2))
        zs[grp] = (z1g, z2g)
    return zs


def _stats_phase0(p, nc, zs, grp):
    """Norms/cosine and the per-element scalar coefficient chain.

    Groups 0-1 run on DVE (fast ramp); later groups on GPSIMD, which is
    otherwise idle, so the recurring stats never steal copy throughput.
    GPSIMD only supports plain tensor_tensor ops (no per-partition
    TensorScalarPtr), so scalar multiplies go through stride-0 broadcasts.
    """
    A = mybir.AluOpType
    gps = grp >= 2
    eng = nc.gpsimd if gps else nc.vector
    st = {"eng": eng, "gps": gps}
    z1g, z2g = zs[grp]

    def wt(tag):
        return p.work.tile([P, D], f32, tag=tag, name=f"w_{tag}_{grp}")

    def sv(tag):
        return p.stat.tile([P, 1], f32, tag=tag, name=f"sv_{tag}_{grp}")

    v1z, v2z, wz = wt("v1z"), wt("v2z"), wt("wz")
    eng.tensor_mul(v1z[:], z1g[:], z1g[:])
    eng.tensor_mul(v2z[:], z2g[:], z2g[:])
    eng.tensor_mul(wz[:], z1g[:], z2g[:])

    s1, s2, dot = sv("s1"), sv("s2"), sv("dot")
    # free-axis reduces are DVE-only (GPSIMD reduces across partitions)
    nc.vector.reduce_sum(s1[:], v1z[:], axis=mybir.AxisListType.X)
    nc.vector.reduce_sum(s2[:], v2z[:], axis=mybir.AxisListType.X)
    nc.vector.reduce_sum(dot[:], wz[:], axis=mybir.AxisListType.X)
    # per-element |z| maxes feed the int8 quantization bound
    mz1, mz2 = sv("mz1"), sv("mz2")
    nc.vector.reduce_max(mz1[:], z1g[:], axis=mybir.AxisListType.X,
                         apply_absolute_value=True)
    nc.vector.reduce_max(mz2[:], z2g[:], axis=mybir.AxisListType.X,
                         apply_absolute_value=True)
    n1, n2 = sv("n1"), sv("n2")
    nc.scalar.sqrt(n1[:], s1[:])
    nc.scalar.sqrt(n2[:], s2[:])
    a, bb = sv("a"), sv("bb")
    nc.vector.reciprocal(a[:], n1[:])
    nc.vector.reciprocal(bb[:], n2[:])
    a2, b2, ab, c = sv("a2"), sv("b2"), sv("ab"), sv("c")
    eng.tensor_mul(a2[:], a[:], a[:])
    eng.tensor_mul(b2[:], bb[:], bb[:])
    eng.tensor_mul(ab[:], a[:], bb[:])
    eng.tensor_mul(c[:], dot[:], ab[:])
    m3c, mc = sv("m3c"), sv("mc")
    if gps:
        # Pool has no tensor_scalar-immediate either; build -3c/-c from adds
        c2, c3 = sv("c2"), sv("c3")
        eng.tensor_add(c2[:], c[:], c[:])
        eng.tensor_add(c3[:], c2[:], c[:])
        eng.tensor_sub(m3c[:], p.zero1[:], c3[:])
        eng.tensor_sub(mc[:], p.zero1[:], c[:])
    else:
        eng.tensor_scalar(m3c[:], c[:], -3.0, None, A.mult)
        eng.tensor_scalar(mc[:], c[:], -1.0, None, A.mult)
    A3B, AB3, A4, B4, A2B2 = sv("A3B"), sv("AB3"), sv("A4"), sv("B4"), sv("A2B2")
    eng.tensor_mul(A3B[:], a2[:], ab[:])
    eng.tensor_mul(AB3[:], b2[:], ab[:])
    eng.tensor_mul(A4[:], a2[:], a2[:])
    eng.tensor_mul(B4[:], b2[:], b2[:])
    eng.tensor_mul(A2B2[:], ab[:], ab[:])
    m3cA4, m3cB4, mcA2B2, mcab = sv("m3cA4"), sv("m3cB4"), sv("mcA2B2"), sv("mcab")
    eng.tensor_mul(m3cA4[:], A4[:], m3c[:])
    eng.tensor_mul(m3cB4[:], B4[:], m3c[:])
    eng.tensor_mul(mcA2B2[:], A2B2[:], mc[:])
    eng.tensor_mul(mcab[:], ab[:], mc[:])

    st.update(z1g=z1g, z2g=z2g, v1z=v1z, v2z=v2z, wz=wz, mz1=mz1, mz2=mz2,
              a2=a2, b2=b2, ab=ab, c=c, m3c=m3c, A3B=A3B, AB3=AB3,
              m3cA4=m3cA4, m3cB4=m3cB4, mcA2B2=mcA2B2, mcab=mcab, wt=wt,
              sv=sv, A4=A4, B4=B4, A2B2=A2B2)
    return st


def _stats_phase1(p, nc, st, grp):
    """rhs rows R0, R1 [128b, 384] in fp16, pre-scaled by sqrt(s).

    Also computes the per-element int8 quantization scale: a guaranteed
    bound |H_rank2| <= max_k(mz1*mr0_k + mz2*mr1_k) from the coefficient
    magnitudes (triangle inequality, no wide reduces), s = 126/bound.
    sqrt(s) is folded into the R coefficients AND the z1h/z2h fp16 copies,
    so PSUM holds s*H and the PSUM->SBUF copy is a plain int8 cast.
    The bound goes to DRAM for the host-side dequantization.
    """
    A = mybir.AluOpType
    gps = st["gps"]
    eng = st["eng"]
    z1g, z2g, wt, sv = st["z1g"], st["z2g"], st["wt"], st["sv"]
    A3B, AB3 = st["A3B"], st["AB3"]
    mz1, mz2 = st["mz1"], st["mz2"]
    R0 = p.rpool.tile([P, KD], f16, tag="R0", name=f"R0_{grp}")
    R1 = p.rpool.tile([P, KD], f16, tag="R1", name=f"R1_{grp}")

    # ---- int8 scale chain ([128,1] ops on the group's stats engine; only
    # mul/add/sub, which are the ops GPSIMD's Pool engine supports) ----
    # |m3cA4| = 3|c|a^4 etc., with |c| = sqrt(c^2) via one ACT op
    cc, cabs, c2a, c3a = sv("cc"), sv("cabs"), sv("c2a"), sv("c3a")
    eng.tensor_mul(cc[:], st["c"][:], st["c"][:])
    nc.scalar.sqrt(cabs[:], cc[:])
    eng.tensor_add(c2a[:], cabs[:], cabs[:])
    eng.tensor_add(c3a[:], c2a[:], cabs[:])
    ab1, ab2, ab3 = sv("ab1"), sv("ab2"), sv("ab3")
    eng.tensor_mul(ab1[:], c3a[:], st["A4"][:])
    eng.tensor_mul(ab2[:], cabs[:], st["A2B2"][:])
    eng.tensor_mul(ab3[:], c3a[:], st["B4"][:])
    tA2 = sv("tA2")   # A3B*mz2
    eng.tensor_mul(tA2[:], A3B[:], mz2[:])
    tA1 = sv("tA1")   # A3B*mz1
    eng.tensor_mul(tA1[:], A3B[:], mz1[:])
    tB2 = sv("tB2")   # AB3*mz2
    eng.tensor_mul(tB2[:], AB3[:], mz2[:])
    tB1 = sv("tB1")   # AB3*mz1
    eng.tensor_mul(tB1[:], AB3[:], mz1[:])
    # mr0_k, mr1_k upper bounds, then bound_k = mz1*mr0_k + mz2*mr1_k
    mr00a, mr00, mr11a, mr11, mr12a, mr12 = (sv("mr00a"), sv("mr00"),
                                             sv("mr11a"), sv("mr11"),
                                             sv("mr12a"), sv("mr12"))
    eng.tensor_mul(mr00a[:], ab1[:], mz1[:])
    eng.tensor_add(mr00[:], mr00a[:], tA2[:])
    eng.tensor_mul(mr11a[:], ab2[:], mz1[:])
    eng.tensor_add(mr11[:], mr11a[:], tB2[:])
    eng.tensor_mul(mr12a[:], ab3[:], mz2[:])
    eng.tensor_add(mr12[:], mr12a[:], tB1[:])
    b0a, b0, b1a, b1, b2a, b2k = (sv("b0a"), sv("b0k"), sv("b1a"), sv("b1k"),
                                  sv("b2a"), sv("b2k"))
    eng.tensor_mul(b0a[:], mr00[:], mz1[:])
    t = sv("bt0")
    eng.tensor_mul(t[:], tA1[:], mz2[:])
    eng.tensor_add(b0[:], b0a[:], t[:])
    eng.tensor_mul(b1a[:], tA1[:], mz1[:])
    t2v = sv("bt1")
    eng.tensor_mul(t2v[:], mr11[:], mz2[:])
    eng.tensor_add(b1[:], b1a[:], t2v[:])
    eng.tensor_mul(b2a[:], tB2[:], mz1[:])
    t3v = sv("bt2")
    eng.tensor_mul(t3v[:], mr12[:], mz2[:])
    eng.tensor_add(b2k[:], b2a[:], t3v[:])
    # sum over k is a valid (<=3x looser) bound and avoids Pool-illegal max
    bnd0, bound = sv("bnd0"), sv("bound")
    eng.tensor_add(bnd0[:], b0[:], b1[:])
    eng.tensor_add(bound[:], bnd0[:], b2k[:])
    inv, s, rs = sv("inv"), sv("s"), sv("rs")
    nc.vector.reciprocal(inv[:], bound[:])
    nc.vector.tensor_scalar(s[:], inv[:], 126.0, None, A.mult)
    nc.scalar.sqrt(rs[:], s[:])
    # dequant scale (bound) to DRAM; ride the group's own stream
    nc.gpsimd.dma_start(p.scl_hw[grp], bound[:])
    # scaled coefficient set: every R term carries exactly one sqrt(s)
    m3cA4s, A3Bs, AB3s, mcA2B2s, m3cB4s = (sv("m3cA4s"), sv("A3Bs"),
                                           sv("AB3s"), sv("mcA2B2s"),
                                           sv("m3cB4s"))
    eng.tensor_mul(m3cA4s[:], st["m3cA4"][:], rs[:])
    eng.tensor_mul(A3Bs[:], A3B[:], rs[:])
    eng.tensor_mul(AB3s[:], AB3[:], rs[:])
    eng.tensor_mul(mcA2B2s[:], st["mcA2B2"][:], rs[:])
    eng.tensor_mul(m3cB4s[:], st["m3cB4"][:], rs[:])
    # fp16 z rows for the lhsT gathers, scaled by sqrt(s) (free: ACT scale)
    z1h = p.zh.tile([P, D], f16, tag="z1h", name=f"z1h_{grp}")
    nc.scalar.mul(z1h[:], z1g[:], rs[:])
    z2h = p.zh.tile([P, D], f16, tag="z2h", name=f"z2h_{grp}")
    nc.scalar.mul(z2h[:], z2g[:], rs[:])

    def ts(out, in0, svt):
        # out = in0 * per-partition scalar svt
        if gps:
            eng.tensor_mul(out, in0, svt[:].broadcast_to([P, D]))
        else:
            eng.tensor_scalar(out, in0, svt[:], None, A.mult)

    def stt(out, in0, svt, in1, tag):
        # out = in0 * svt + in1
        if gps:
            tmp = wt(tag)
            eng.tensor_mul(tmp[:], in0, svt[:].broadcast_to([P, D]))
            eng.tensor_add(out, tmp[:], in1)
        else:
            eng.scalar_tensor_tensor(out, in0, svt[:], in1, A.mult, A.add)

    t0 = wt("t0")
    # k=0 (H11): r0 = m3cA4*z1 + A3B*z2 ; r1 = A3B*z1
    ts(t0[:], z2g[:], A3Bs)
    stt(R0[:, 0:D], z1g[:], m3cA4s, t0[:], "p1a")
    ts(R1[:, 0:D], z1g[:], A3Bs)
    # k=1 (H12): r0 = A3B*z1 ; r1 = mcA2B2*z1 + AB3*z2
    ts(R0[:, D:2 * D], z1g[:], A3Bs)
    t1 = wt("t1")
    ts(t1[:], z2g[:], AB3s)
    stt(R1[:, D:2 * D], z1g[:], mcA2B2s, t1[:], "p1b")
    # k=2 (H22): r0 = AB3*z2 ; r1 = AB3*z1 + m3cB4*z2
    ts(R0[:, 2 * D:3 * D], z2g[:], AB3s)
    t2 = wt("t2")
    ts(t2[:], z2g[:], m3cB4s)
    stt(R1[:, 2 * D:3 * D], z1g[:], AB3s, t2[:], "p1c")
    st.update(R0=R0, R1=R1, z1h=z1h, z2h=z2h)


def _stats_phase2(p, nc, st, grp, diag_hw):
    """Final diagonal values, batch-major [128b, 3*128i]; DMA'd out as f32.

    Host overwrites out[k, b, i, i] with these.
    """
    A = mybir.AluOpType
    gps = st["gps"]
    eng = st["eng"]
    wt = st["wt"]

    def sv(tag):
        return p.stat.tile([P, 1], f32, tag=tag, name=f"sv_{tag}_{grp}")

    v1z, v2z, wz = st["v1z"], st["v2z"], st["wz"]
    a2, b2, ab, c, m3c = st["a2"], st["b2"], st["ab"], st["c"], st["m3c"]
    dall = p.dpool.tile([P, KD], f32, tag="dall", name=f"dall_{grp}")

    def bc(svt):
        return svt[:].broadcast_to([P, D])

    twoabw = wt("twoabw")
    if gps:
        ab2 = sv("ab2")
        eng.tensor_add(ab2[:], ab[:], ab[:])
        eng.tensor_mul(twoabw[:], wz[:], bc(ab2))
        # d11 = a2*(c + 2ab*wz + m3c*a2*v1z)
        u1, u2, u2c = wt("u1"), wt("u2"), wt("u2c")
        pa = sv("pa")
        eng.tensor_mul(pa[:], a2[:], m3c[:])
        eng.tensor_mul(u1[:], v1z[:], bc(pa))
        eng.tensor_add(u2[:], u1[:], twoabw[:])
        eng.tensor_add(u2c[:], u2[:], bc(c))
        eng.tensor_mul(dall[:, 0:D], u2c[:], bc(a2))
        # d12 = ab*(a2*v1z + b2*v2z + mcab*wz - 1)
        w1, w2, w2b, w3, w3b, w4 = (wt("w1"), wt("w2"), wt("w2b"), wt("w3"),
                                    wt("w3b"), wt("w4"))
        eng.tensor_mul(w1[:], v1z[:], bc(a2))
        eng.tensor_mul(w2[:], v2z[:], bc(b2))
        eng.tensor_add(w2b[:], w2[:], w1[:])
        eng.tensor_mul(w3[:], wz[:], bc(st["mcab"]))
        eng.tensor_add(w3b[:], w3[:], w2b[:])
        eng.tensor_mul(w4[:], w3b[:], bc(ab))
        eng.tensor_sub(dall[:, D:2 * D], w4[:], bc(ab))
        # d22 = b2*(c + 2ab*wz + m3c*b2*v2z)
        u3, u4, u4c = wt("u3"), wt("u4"), wt("u4c")
        pb = sv("pb")
        eng.tensor_mul(pb[:], b2[:], m3c[:])
        eng.tensor_mul(u3[:], v2z[:], bc(pb))
        eng.tensor_add(u4[:], u3[:], twoabw[:])
        eng.tensor_add(u4c[:], u4[:], bc(c))
        eng.tensor_mul(dall[:, 2 * D:3 * D], u4c[:], bc(b2))
        # diag DMA rides the GPS SWDGE stream: the data was just produced
        # here, so it never blocks the sync ring behind a slow stats chain
        nc.gpsimd.dma_start(diag_hw[grp], dall[:])
    else:
        eng.tensor_scalar(twoabw[:], wz[:], ab[:], 2.0, A.mult, A.mult)
        # d11 = a2*(c + 2ab*wz + m3c*a2*v1z)
        u1, u2 = wt("u1"), wt("u2")
        eng.tensor_scalar(u1[:], v1z[:], a2[:], m3c[:], A.mult, A.mult)
        eng.tensor_add(u2[:], u1[:], twoabw[:])
        eng.tensor_scalar(dall[:, 0:D], u2[:], c[:], a2[:], A.add, A.mult)
        # d12 = ab*(a2*v1z + b2*v2z + mcab*wz - 1)
        w1, w2, w3 = wt("w1"), wt("w2"), wt("w3")
        eng.tensor_scalar(w1[:], v1z[:], a2[:], None, A.mult)
        eng.scalar_tensor_tensor(w2[:], v2z[:], b2[:], w1[:], A.mult, A.add)
        eng.scalar_tensor_tensor(w3[:], wz[:], st["mcab"][:], w2[:],
                                 A.mult, A.add)
        eng.tensor_scalar(dall[:, D:2 * D], w3[:], -1.0, ab[:], A.add, A.mult)
        # d22 = b2*(c + 2ab*wz + m3c*b2*v2z)
        u3, u4 = wt("u3"), wt("u4")
        eng.tensor_scalar(u3[:], v2z[:], b2[:], m3c[:], A.mult, A.mult)
        eng.tensor_add(u4[:], u3[:], twoabw[:])
        eng.tensor_scalar(dall[:, 2 * D:3 * D], u4[:], c[:], b2[:],
                          A.add, A.mult)
        nc.gpsimd.dma_start(diag_hw[grp], dall[:])


def _emit_group_gathers(p, nc, st, grp):
    """Operand gathers for a WHOLE group (128 elements), on the sync ring.

    A dma_start dispatch costs ~0.6us of sequencer time regardless of size,
    so gathering per group (8 dmas) instead of per chunk (32) keeps the
    sync sequencer free for the output writes. Emitted 1.5 chunks ahead of
    the group's first matmul; sources (z1h/R of that group) are computed a
    full group earlier, so the dispatch never blocks the ring.
    """
    HF = P // 2               # 64 elements per interleaved half
    ZI = p.zi.tile([P, HF * D], f16, tag="ZI", name=f"ZI_{grp}")
    RI = p.ri.tile([P, HF * KD], f16, tag="RI", name=f"RI_{grp}")
    z1h, z2h, R0, R1 = st["z1h"], st["z2h"], st["R0"], st["R1"]
    for half in range(2):
        hb = HF * half
        pp = 32 * half
        nc.gpsimd.dma_start(ZI[pp:pp + 1, :], z1h[hb:hb + HF, :])
        nc.gpsimd.dma_start(ZI[pp + 1:pp + 2, :], z2h[hb:hb + HF, :])
        nc.gpsimd.dma_start(RI[pp:pp + 1, :], R0[hb:hb + HF, :])
        nc.gpsimd.dma_start(RI[pp + 1:pp + 2, :], R1[hb:hb + HF, :])
    return ZI, RI


def _emit_chunk(p, nc, ZI, RI, out_hw, grp, ch, qctr):
    """G elements (matmul + quad PSUM->fp16 copy) + output DMA."""
    e0 = grp * P + ch * G     # global element base for this chunk
    ci = grp * NCH + ch
    GS = 8 if ci == 0 else 16
    for sub in range(G // GS):
        STG = p.stage.tile([P, GS * KD], i8, tag="STG",
                           name=f"STG_{grp}_{ch}_{sub}")
        for t in range(GS // QB):
            pt = p.mmp.tile([P, QB * 512], f32, tag="pt",
                            name=f"pt_{grp}_{ch}_{sub}_{t}")
            for slot in range(QB):
                s = sub * GS + t * QB + slot   # local element in chunk
                half = s % 2                   # PE quadrant alternation
                ff = ch * (G // 2) + s // 2    # column within group tile
                pp = 32 * half
                lhsT = ZI[pp:pp + 2, ff * D:(ff + 1) * D]
                rhs = RI[pp:pp + 2, ff * KD:(ff + 1) * KD]
                nc.tensor.matmul(pt[:, slot * 512:slot * 512 + KD],
                                 lhsT, rhs, start=True, stop=True)
            src = pt[:].rearrange("p (q c) -> p q c", c=512)[:, :, 0:KD]
            dst = STG[:, t * QB * KD:(t + 1) * QB * KD].rearrange(
                "p (q c) -> p q c", c=KD)
            if qctr[0] % 2 == 0:
                nc.vector.tensor_copy(dst, src)
            else:
                nc.scalar.copy(dst, src)
            qctr[0] += 1
        es = e0 + sub * GS
        nc.sync.dma_start(out_hw[:, es * KD:(es + GS) * KD], STG[:])


def _build_body(ctx, tc, z1, z2, out_hw, diag_hw, scl_hw):
    nc = tc.nc
    p = _make_pools(ctx, tc)
    p.scl_hw = scl_hw
    p.zero1 = p.const.tile([P, 1], f32, tag="zero1", name="zero1")
    nc.vector.memset(p.zero1[:], 0.0)

    # Stats run TWO groups ahead of consumption (phase0 of g+2 during group
    # g; phases 1-2 of g+1 at the start of group g) so the ~25us stats chain
    # latency (GPS serial ops + cross-engine hops queued behind copies)
    # hides under ~70us of compute. Stats phases are emitted BEFORE the
    # chunk body so their DVE/ACT hops enqueue ahead of that chunk's copies.
    qctr = [0]
    zs = _load_all_z(p, nc, z1, z2)
    sts = {0: _stats_phase0(p, nc, zs, 0)}
    _stats_phase1(p, nc, sts[0], 0)
    gath = {0: _emit_group_gathers(p, nc, sts[0], 0)}
    _stats_phase2(p, nc, sts[0], 0, diag_hw)
    if GROUPS > 1:
        # group 1 also runs on DVE, in the ramp window before copies
        # saturate it, so R(g1) is ready long before the first boundary
        sts[1] = _stats_phase0(p, nc, zs, 1)
        _stats_phase1(p, nc, sts[1], 1)
        _stats_phase2(p, nc, sts[1], 1, diag_hw)
    TOT = GROUPS * NCH
    for ci in range(TOT):
        grp, ch = divmod(ci, NCH)
        if ch == 2 and grp + 1 < GROUPS:
            # next group's gathers: R/z1h computed by now, so the sync
            # dispatch is wait-free, 1.5 chunks ahead of first use
            gath[grp + 1] = _emit_group_gathers(p, nc, sts[grp + 1], grp + 1)
        ZI, RI = gath[grp]
        _emit_chunk(p, nc, ZI, RI, out_hw, grp, ch, qctr)
        # stats for group g+2 go AFTER the chunk body: their DVE/ACT hops
        # then queue behind this chunk's copies instead of blocking them
        # while waiting on the GPS chain; the 2-group prefetch absorbs the
        # added latency
        if grp + 2 < GROUPS:
            if ch == 0:
                sts[grp + 2] = _stats_phase0(p, nc, zs, grp + 2)
            elif ch == 1:
                _stats_phase1(p, nc, sts[grp + 2], grp + 2)
            elif ch == 2:
                _stats_phase2(p, nc, sts[grp + 2], grp + 2, diag_hw)


def build_kernel():
    nc = bacc.Bacc("TRN2", target_bir_lowering=False, debug=False)
    z1 = nc.dram_tensor("z1", [B_SH, D], f32, kind="ExternalInput").ap()
    z2 = nc.dram_tensor("z2", [B_SH, D], f32, kind="ExternalInput").ap()
    # packed rank-2 output: [i partition, (b, k, j) free], fp16
    out_hw = nc.dram_tensor("out", [P, B_SH * KD], i8,
                            kind="ExternalOutput").ap()
    scl_hw = nc.dram_tensor("scl", [GROUPS, P, 1], f32,
                            kind="ExternalOutput").ap()
    # final diagonals: [group, b-partition (interleaved), (k, i) free], f32
    diag_hw = nc.dram_tensor("diag", [GROUPS, P, KD], f32,
                             kind="ExternalOutput").ap()
    with tile.TileContext(nc) as tc:
        with ExitStack() as ctx:
            _build_body(ctx, tc, z1, z2, out_hw, diag_hw, scl_hw)
    nc.compile()
    return nc


_NC_CACHE = None


def _get_nc():
    global _NC_CACHE
    if _NC_CACHE is None:
        _NC_CACHE = build_kernel()
    return _NC_CACHE


def _assemble(out_c, diag_c, scl_c, dst):
    """Unpack one core's HW tensors into dst [3, B_SH, D, D] f32."""
    # out_c [128 i, B_SH*384] int8 -> [i, b, k, j] -> [k, b, i, j]; dequant
    # with the per-element scale bound/126 (scl rows are partition-ordered)
    h = out_c.reshape(P, B_SH, 3, D).transpose(2, 1, 0, 3)
    scale = (scl_c.reshape(GROUPS, P)[:, PART_OF_ELEM].reshape(B_SH)
             / np.float32(126.0))
    np.multiply(h, scale[None, :, None, None], out=dst, dtype=np.float32)
    # diag_c [GROUPS, 128 p, 3*128] f32; partition p holds group element
    # E_OF_P... i.e. ordered element e sits at partition P_OF_E[e]
    dv = diag_c[:, PART_OF_ELEM, :].reshape(GROUPS, P, 3, D).transpose(
        2, 0, 1, 3).reshape(3, B_SH, D)
    dst.reshape(3, B_SH, D * D)[:, :, ::D + 1] = dv


def kernel(z1, z2):
    nc = _get_nc()
    z1 = np.ascontiguousarray(np.asarray(z1, dtype=np.float32))
    z2 = np.ascontiguousarray(np.asarray(z2, dtype=np.float32))
    in_maps = [
        {"z1": z1[c * B_SH:(c + 1) * B_SH], "z2": z2[c * B_SH:(c + 1) * B_SH]}
        for c in range(N_CORES)
    ]
    res = run_bass_kernel_spmd(nc, in_maps, core_ids=list(range(N_CORES)))
    full = np.empty((3, B_FULL, D, D), dtype=np.float32)
    for c in range(N_CORES):
        _assemble(res.results[c]["out"], res.results[c]["diag"],
                  res.results[c]["scl"], full[:, c * B_SH:(c + 1) * B_SH])
    return full


# revision 40
# speedup vs baseline: 1.2635x; 1.0078x over previous
"""Trainium2 Bass kernel for ArccosHessianCalculator (v3).

Math: for each batch element b (z1, z2 are [B, D] with D = 128):
  a = 1/|z1|, bb = 1/|z2|, c = cos = <z1u, z2u>
  Each Hessian block H_k is rank-2 plus a diagonal term:
      H_k(b) = z1 * r0_k(b)^T + z2 * r1_k(b)^T + diag
  with the normalization / cosine factors folded into r0/r1:
      k=0 (H11): r0 = -3c*a^4*z1 + a^3 b*z2          r1 = a^3 b*z1
      k=1 (H12): r0 = a^3 b*z1                        r1 = -c*a^2 b^2*z1 + a b^3*z2
      k=2 (H22): r0 = a b^3*z2                        r1 = a b^3*z1 - 3c*b^4*z2

Device strategy (per core, batch shard of 512):
  - TensorE: one K=2 matmul per element, lhsT = [z1(b); z2(b)] ([2,128] fp16),
    rhs = [r0 | r1] blocks ([2, 384] fp16) -> PSUM f32 [128, 384]. Elements
    alternate PE quadrant rows ({0,1} / {32,33}) so LDWEIGHTS for element
    e+1 overlaps the stream of element e.
  - The rank-2 part goes to DRAM in its native layout as fp16:
    out_hw [128 i, B_SH*384 (b,k,j)] -- per-partition contiguous runs, so
    the output DMA runs at descriptor line rate; fp16 halves HBM traffic.
  - Final diagonals are computed in closed form per group ([128 b, 3*128 i])
    into a tiny f32 side tensor and spliced on the HOST (no predicated copy
    on device).
  - PSUM is organized as 4-bank super-tiles: 4 matmuls fill one, then ONE
    strided copy drains all 4 to fp16 SBUF, amortizing per-instruction
    overhead. Copies alternate DVE / ACT.
  - Stats run on GPSIMD for groups >= 1 (group 0 on DVE for a fast ramp);
    the R0/R1 builds stay on DVE (fp16 writes), sqrt on ACT, recip on DVE.
  - Batch rows are loaded interleaved (even elements -> partitions 0..63,
    odd -> 64..127) so per-chunk gathers are plain partition slices.
  - Host: one reshape/transpose view + diagonal stride-trick splice + cast.
"""

import numpy as np
from contextlib import ExitStack

import concourse.bass as bass
import concourse.tile as tile
from concourse import bacc, mybir
from concourse.bass_utils import run_bass_kernel_spmd

N_CORES = 8
B_FULL = 4096
D = 128
B_SH = B_FULL // N_CORES  # 512 batch elements per core
P = 128                   # SBUF partitions
KD = 3 * D                # 384: three H blocks side by side
F = 16                    # elements per gather partition-row
G = 32                    # elements per chunk
GROUPS = B_SH // P        # 4 stats groups of 128 elements
NCH = P // G              # 4 chunks per group
QB = 2                    # elements per PSUM super-tile (2 banks)

f32 = mybir.dt.float32
f16 = mybir.dt.float16
i8 = mybir.dt.int8

# interleaved element order within a group: partition p holds group element
# 2p (p < 64) or 2(p-64)+1 (p >= 64)
ELEM_OF_PART = np.concatenate([np.arange(0, P, 2), np.arange(1, P, 2)])
PART_OF_ELEM = np.argsort(ELEM_OF_PART)


class _Pools:
    pass


def _make_pools(ctx, tc):
    p = _Pools()
    p.const = ctx.enter_context(tc.tile_pool(name="const", bufs=1))
    p.zg = ctx.enter_context(tc.tile_pool(name="zg", bufs=1))
    p.zh = ctx.enter_context(tc.tile_pool(name="zh", bufs=2))
    p.work = ctx.enter_context(tc.tile_pool(name="work", bufs=2))
    p.stat = ctx.enter_context(tc.tile_pool(name="stat", bufs=3))
    p.rpool = ctx.enter_context(tc.tile_pool(name="rpool", bufs=2))
    p.dpool = ctx.enter_context(tc.tile_pool(name="dpool", bufs=2))
    p.zi = ctx.enter_context(tc.tile_pool(name="zi", bufs=2))
    p.ri = ctx.enter_context(tc.tile_pool(name="ri", bufs=2))
    p.stage = ctx.enter_context(tc.tile_pool(name="stage", bufs=3))
    p.mmp = ctx.enter_context(tc.tile_pool(name="mmp", bufs=4, space="PSUM"))
    return p


def _load_all_z(p, nc, z1, z2):
    """Prefetch every group's z tiles at t=0 (tiny: 512B/partition each).

    Keeping all loads at the head of their queues means no stats-chain wait
    can ever delay a z load, and the sync ring only ever carries outputs.
    """
    zs = {}
    for grp in range(GROUPS):
        b0 = grp * P
        ldma = nc.sync if grp == 0 else nc.gpsimd
        # interleaved row order: partition p <- batch row 2p / 2(p-64)+1
        z1g = p.zg.tile([P, D], f32, tag=f"z1g{grp}", name=f"z1g_{grp}")
        ldma.dma_start(z1g[:], z1[b0:b0 + P, :].rearrange(
            "(f two) d -> two f d", two=2))
        z2g = p.zg.tile([P, D], f32, tag=f"z2g{grp}", name=f"z2g_{grp}")
        ldma.dma_start(z2g[:], z2[b0:b0 + P, :].rearrange(
            "(f two) d -> two f d", two=2))
        zs[grp] = (z1g, z2g)
    return zs


def _stats_phase0(p, nc, zs, grp):
    """Norms/cosine and the per-element scalar coefficient chain.

    Groups 0-1 run on DVE (fast ramp); later groups on GPSIMD, which is
    otherwise idle, so the recurring stats never steal copy throughput.
    GPSIMD only supports plain tensor_tensor ops (no per-partition
    TensorScalarPtr), so scalar multiplies go through stride-0 broadcasts.
    """
    A = mybir.AluOpType
    gps = grp >= 2
    eng = nc.gpsimd if gps else nc.vector
    st = {"eng": eng, "gps": gps}
    z1g, z2g = zs[grp]

    def wt(tag):
        return p.work.tile([P, D], f32, tag=tag, name=f"w_{tag}_{grp}")

    def sv(tag):
        return p.stat.tile([P, 1], f32, tag=tag, name=f"sv_{tag}_{grp}")

    v1z, v2z, wz = wt("v1z"), wt("v2z"), wt("wz")
    eng.tensor_mul(v1z[:], z1g[:], z1g[:])
    eng.tensor_mul(v2z[:], z2g[:], z2g[:])
    eng.tensor_mul(wz[:], z1g[:], z2g[:])

    s1, s2, dot = sv("s1"), sv("s2"), sv("dot")
    # free-axis reduces are DVE-only (GPSIMD reduces across partitions)
    nc.vector.reduce_sum(s1[:], v1z[:], axis=mybir.AxisListType.X)
    nc.vector.reduce_sum(s2[:], v2z[:], axis=mybir.AxisListType.X)
    nc.vector.reduce_sum(dot[:], wz[:], axis=mybir.AxisListType.X)
    # per-element |z| maxes feed the int8 quantization bound
    mz1, mz2 = sv("mz1"), sv("mz2")
    nc.vector.reduce_max(mz1[:], z1g[:], axis=mybir.AxisListType.X,
                         apply_absolute_value=True)
    nc.vector.reduce_max(mz2[:], z2g[:], axis=mybir.AxisListType.X,
                         apply_absolute_value=True)
    n1, n2 = sv("n1"), sv("n2")
    nc.scalar.sqrt(n1[:], s1[:])
    nc.scalar.sqrt(n2[:], s2[:])
    a, bb = sv("a"), sv("bb")
    nc.vector.reciprocal(a[:], n1[:])
    nc.vector.reciprocal(bb[:], n2[:])
    a2, b2, ab, c = sv("a2"), sv("b2"), sv("ab"), sv("c")
    eng.tensor_mul(a2[:], a[:], a[:])
    eng.tensor_mul(b2[:], bb[:], bb[:])
    eng.tensor_mul(ab[:], a[:], bb[:])
    eng.tensor_mul(c[:], dot[:], ab[:])
    m3c, mc = sv("m3c"), sv("mc")
    if gps:
        # Pool has no tensor_scalar-immediate either; build -3c/-c from adds
        c2, c3 = sv("c2"), sv("c3")
        eng.tensor_add(c2[:], c[:], c[:])
        eng.tensor_add(c3[:], c2[:], c[:])
        eng.tensor_sub(m3c[:], p.zero1[:], c3[:])
        eng.tensor_sub(mc[:], p.zero1[:], c[:])
    else:
        eng.tensor_scalar(m3c[:], c[:], -3.0, None, A.mult)
        eng.tensor_scalar(mc[:], c[:], -1.0, None, A.mult)
    A3B, AB3, A4, B4, A2B2 = sv("A3B"), sv("AB3"), sv("A4"), sv("B4"), sv("A2B2")
    eng.tensor_mul(A3B[:], a2[:], ab[:])
    eng.tensor_mul(AB3[:], b2[:], ab[:])
    eng.tensor_mul(A4[:], a2[:], a2[:])
    eng.tensor_mul(B4[:], b2[:], b2[:])
    eng.tensor_mul(A2B2[:], ab[:], ab[:])
    m3cA4, m3cB4, mcA2B2, mcab = sv("m3cA4"), sv("m3cB4"), sv("mcA2B2"), sv("mcab")
    eng.tensor_mul(m3cA4[:], A4[:], m3c[:])
    eng.tensor_mul(m3cB4[:], B4[:], m3c[:])
    eng.tensor_mul(mcA2B2[:], A2B2[:], mc[:])
    eng.tensor_mul(mcab[:], ab[:], mc[:])

    st.update(z1g=z1g, z2g=z2g, v1z=v1z, v2z=v2z, wz=wz, mz1=mz1, mz2=mz2,
              a2=a2, b2=b2, ab=ab, c=c, m3c=m3c, A3B=A3B, AB3=AB3,
              m3cA4=m3cA4, m3cB4=m3cB4, mcA2B2=mcA2B2, mcab=mcab, wt=wt,
              sv=sv, A4=A4, B4=B4, A2B2=A2B2)
    return st


def _stats_phase1(p, nc, st, grp):
    """rhs rows R0, R1 [128b, 384] in fp16, pre-scaled by sqrt(s).

    Also computes the per-element int8 quantization scale: a guaranteed
    bound |H_rank2| <= max_k(mz1*mr0_k + mz2*mr1_k) from the coefficient
    magnitudes (triangle inequality, no wide reduces), s = 126/bound.
    sqrt(s) is folded into the R coefficients AND the z1h/z2h fp16 copies,
    so PSUM holds s*H and the PSUM->SBUF copy is a plain int8 cast.
    The bound goes to DRAM for the host-side dequantization.
    """
    A = mybir.AluOpType
    gps = st["gps"]
    eng = st["eng"]
    z1g, z2g, wt, sv = st["z1g"], st["z2g"], st["wt"], st["sv"]
    A3B, AB3 = st["A3B"], st["AB3"]
    mz1, mz2 = st["mz1"], st["mz2"]
    R0 = p.rpool.tile([P, KD], f16, tag="R0", name=f"R0_{grp}")
    R1 = p.rpool.tile([P, KD], f16, tag="R1", name=f"R1_{grp}")

    # ---- int8 scale chain ([128,1] ops on the group's stats engine; only
    # mul/add/sub, which are the ops GPSIMD's Pool engine supports) ----
    # |m3cA4| = 3|c|a^4 etc., with |c| = sqrt(c^2) via one ACT op
    cc, cabs, c2a, c3a = sv("cc"), sv("cabs"), sv("c2a"), sv("c3a")
    eng.tensor_mul(cc[:], st["c"][:], st["c"][:])
    nc.scalar.sqrt(cabs[:], cc[:])
    eng.tensor_add(c2a[:], cabs[:], cabs[:])
    eng.tensor_add(c3a[:], c2a[:], cabs[:])
    ab1, ab2, ab3 = sv("ab1"), sv("ab2"), sv("ab3")
    eng.tensor_mul(ab1[:], c3a[:], st["A4"][:])
    eng.tensor_mul(ab2[:], cabs[:], st["A2B2"][:])
    eng.tensor_mul(ab3[:], c3a[:], st["B4"][:])
    tA2 = sv("tA2")   # A3B*mz2
    eng.tensor_mul(tA2[:], A3B[:], mz2[:])
    tA1 = sv("tA1")   # A3B*mz1
    eng.tensor_mul(tA1[:], A3B[:], mz1[:])
    tB2 = sv("tB2")   # AB3*mz2
    eng.tensor_mul(tB2[:], AB3[:], mz2[:])
    tB1 = sv("tB1")   # AB3*mz1
    eng.tensor_mul(tB1[:], AB3[:], mz1[:])
    # mr0_k, mr1_k upper bounds, then bound_k = mz1*mr0_k + mz2*mr1_k
    mr00a, mr00, mr11a, mr11, mr12a, mr12 = (sv("mr00a"), sv("mr00"),
                                             sv("mr11a"), sv("mr11"),
                                             sv("mr12a"), sv("mr12"))
    eng.tensor_mul(mr00a[:], ab1[:], mz1[:])
    eng.tensor_add(mr00[:], mr00a[:], tA2[:])
    eng.tensor_mul(mr11a[:], ab2[:], mz1[:])
    eng.tensor_add(mr11[:], mr11a[:], tB2[:])
    eng.tensor_mul(mr12a[:], ab3[:], mz2[:])
    eng.tensor_add(mr12[:], mr12a[:], tB1[:])
    b0a, b0, b1a, b1, b2a, b2k = (sv("b0a"), sv("b0k"), sv("b1a"), sv("b1k"),
                                  sv("b2a"), sv("b2k"))
    eng.tensor_mul(b0a[:], mr00[:], mz1[:])
    t = sv("bt0")
    eng.tensor_mul(t[:], tA1[:], mz2[:])
    eng.tensor_add(b0[:], b0a[:], t[:])
    eng.tensor_mul(b1a[:], tA1[:], mz1[:])
    t2v = sv("bt1")
    eng.tensor_mul(t2v[:], mr11[:], mz2[:])
    eng.tensor_add(b1[:], b1a[:], t2v[:])
    eng.tensor_mul(b2a[:], tB2[:], mz1[:])
    t3v = sv("bt2")
    eng.tensor_mul(t3v[:], mr12[:], mz2[:])
    eng.tensor_add(b2k[:], b2a[:], t3v[:])
    # sum over k is a valid (<=3x looser) bound and avoids Pool-illegal max
    bnd0, bound = sv("bnd0"), sv("bound")
    eng.tensor_add(bnd0[:], b0[:], b1[:])
    eng.tensor_add(bound[:], bnd0[:], b2k[:])
    inv, s, rs = sv("inv"), sv("s"), sv("rs")
    nc.vector.reciprocal(inv[:], bound[:])
    nc.vector.tensor_scalar(s[:], inv[:], 126.0, None, A.mult)
    nc.scalar.sqrt(rs[:], s[:])
    # dequant scale (bound) to DRAM; ride the group's own stream
    nc.gpsimd.dma_start(p.scl_hw[grp], bound[:])
    # scaled coefficient set: every R term carries exactly one sqrt(s)
    m3cA4s, A3Bs, AB3s, mcA2B2s, m3cB4s = (sv("m3cA4s"), sv("A3Bs"),
                                           sv("AB3s"), sv("mcA2B2s"),
                                           sv("m3cB4s"))
    eng.tensor_mul(m3cA4s[:], st["m3cA4"][:], rs[:])
    eng.tensor_mul(A3Bs[:], A3B[:], rs[:])
    eng.tensor_mul(AB3s[:], AB3[:], rs[:])
    eng.tensor_mul(mcA2B2s[:], st["mcA2B2"][:], rs[:])
    eng.tensor_mul(m3cB4s[:], st["m3cB4"][:], rs[:])
    # fp16 z rows for the lhsT gathers, scaled by sqrt(s) (free: ACT scale)
    z1h = p.zh.tile([P, D], f16, tag="z1h", name=f"z1h_{grp}")
    nc.scalar.mul(z1h[:], z1g[:], rs[:])
    z2h = p.zh.tile([P, D], f16, tag="z2h", name=f"z2h_{grp}")
    nc.scalar.mul(z2h[:], z2g[:], rs[:])

    def ts(out, in0, svt):
        # out = in0 * per-partition scalar svt
        if gps:
            eng.tensor_mul(out, in0, svt[:].broadcast_to([P, D]))
        else:
            eng.tensor_scalar(out, in0, svt[:], None, A.mult)

    def stt(out, in0, svt, in1, tag):
        # out = in0 * svt + in1
        if gps:
            tmp = wt(tag)
            eng.tensor_mul(tmp[:], in0, svt[:].broadcast_to([P, D]))
            eng.tensor_add(out, tmp[:], in1)
        else:
            eng.scalar_tensor_tensor(out, in0, svt[:], in1, A.mult, A.add)

    t0 = wt("t0")
    # k=0 (H11): r0 = m3cA4*z1 + A3B*z2 ; r1 = A3B*z1
    ts(t0[:], z2g[:], A3Bs)
    stt(R0[:, 0:D], z1g[:], m3cA4s, t0[:], "p1a")
    ts(R1[:, 0:D], z1g[:], A3Bs)
    # k=1 (H12): r0 = A3B*z1 ; r1 = mcA2B2*z1 + AB3*z2
    ts(R0[:, D:2 * D], z1g[:], A3Bs)
    t1 = wt("t1")
    ts(t1[:], z2g[:], AB3s)
    stt(R1[:, D:2 * D], z1g[:], mcA2B2s, t1[:], "p1b")
    # k=2 (H22): r0 = AB3*z2 ; r1 = AB3*z1 + m3cB4*z2
    ts(R0[:, 2 * D:3 * D], z2g[:], AB3s)
    t2 = wt("t2")
    ts(t2[:], z2g[:], m3cB4s)
    stt(R1[:, 2 * D:3 * D], z1g[:], AB3s, t2[:], "p1c")
    st.update(R0=R0, R1=R1, z1h=z1h, z2h=z2h)


def _stats_phase2(p, nc, st, grp, diag_hw):
    """Final diagonal values, batch-major [128b, 3*128i]; DMA'd out as f32.

    Host overwrites out[k, b, i, i] with these.
    """
    A = mybir.AluOpType
    gps = st["gps"]
    eng = st["eng"]
    wt = st["wt"]

    def sv(tag):
        return p.stat.tile([P, 1], f32, tag=tag, name=f"sv_{tag}_{grp}")

    v1z, v2z, wz = st["v1z"], st["v2z"], st["wz"]
    a2, b2, ab, c, m3c = st["a2"], st["b2"], st["ab"], st["c"], st["m3c"]
    dall = p.dpool.tile([P, KD], f32, tag="dall", name=f"dall_{grp}")

    def bc(svt):
        return svt[:].broadcast_to([P, D])

    twoabw = wt("twoabw")
    if gps:
        ab2 = sv("ab2")
        eng.tensor_add(ab2[:], ab[:], ab[:])
        eng.tensor_mul(twoabw[:], wz[:], bc(ab2))
        # d11 = a2*(c + 2ab*wz + m3c*a2*v1z)
        u1, u2, u2c = wt("u1"), wt("u2"), wt("u2c")
        pa = sv("pa")
        eng.tensor_mul(pa[:], a2[:], m3c[:])
        eng.tensor_mul(u1[:], v1z[:], bc(pa))
        eng.tensor_add(u2[:], u1[:], twoabw[:])
        eng.tensor_add(u2c[:], u2[:], bc(c))
        eng.tensor_mul(dall[:, 0:D], u2c[:], bc(a2))
        # d12 = ab*(a2*v1z + b2*v2z + mcab*wz - 1)
        w1, w2, w2b, w3, w3b, w4 = (wt("w1"), wt("w2"), wt("w2b"), wt("w3"),
                                    wt("w3b"), wt("w4"))
        eng.tensor_mul(w1[:], v1z[:], bc(a2))
        eng.tensor_mul(w2[:], v2z[:], bc(b2))
        eng.tensor_add(w2b[:], w2[:], w1[:])
        eng.tensor_mul(w3[:], wz[:], bc(st["mcab"]))
        eng.tensor_add(w3b[:], w3[:], w2b[:])
        eng.tensor_mul(w4[:], w3b[:], bc(ab))
        eng.tensor_sub(dall[:, D:2 * D], w4[:], bc(ab))
        # d22 = b2*(c + 2ab*wz + m3c*b2*v2z)
        u3, u4, u4c = wt("u3"), wt("u4"), wt("u4c")
        pb = sv("pb")
        eng.tensor_mul(pb[:], b2[:], m3c[:])
        eng.tensor_mul(u3[:], v2z[:], bc(pb))
        eng.tensor_add(u4[:], u3[:], twoabw[:])
        eng.tensor_add(u4c[:], u4[:], bc(c))
        eng.tensor_mul(dall[:, 2 * D:3 * D], u4c[:], bc(b2))
        # diag DMA rides the GPS SWDGE stream: the data was just produced
        # here, so it never blocks the sync ring behind a slow stats chain
        nc.gpsimd.dma_start(diag_hw[grp], dall[:])
    else:
        eng.tensor_scalar(twoabw[:], wz[:], ab[:], 2.0, A.mult, A.mult)
        # d11 = a2*(c + 2ab*wz + m3c*a2*v1z)
        u1, u2 = wt("u1"), wt("u2")
        eng.tensor_scalar(u1[:], v1z[:], a2[:], m3c[:], A.mult, A.mult)
        eng.tensor_add(u2[:], u1[:], twoabw[:])
        eng.tensor_scalar(dall[:, 0:D], u2[:], c[:], a2[:], A.add, A.mult)
        # d12 = ab*(a2*v1z + b2*v2z + mcab*wz - 1)
        w1, w2, w3 = wt("w1"), wt("w2"), wt("w3")
        eng.tensor_scalar(w1[:], v1z[:], a2[:], None, A.mult)
        eng.scalar_tensor_tensor(w2[:], v2z[:], b2[:], w1[:], A.mult, A.add)
        eng.scalar_tensor_tensor(w3[:], wz[:], st["mcab"][:], w2[:],
                                 A.mult, A.add)
        eng.tensor_scalar(dall[:, D:2 * D], w3[:], -1.0, ab[:], A.add, A.mult)
        # d22 = b2*(c + 2ab*wz + m3c*b2*v2z)
        u3, u4 = wt("u3"), wt("u4")
        eng.tensor_scalar(u3[:], v2z[:], b2[:], m3c[:], A.mult, A.mult)
        eng.tensor_add(u4[:], u3[:], twoabw[:])
        eng.tensor_scalar(dall[:, 2 * D:3 * D], u4[:], c[:], b2[:],
                          A.add, A.mult)
        nc.gpsimd.dma_start(diag_hw[grp], dall[:])


def _emit_group_gathers(p, nc, st, grp):
    """Operand gathers for a WHOLE group (128 elements), on the sync ring.

    A dma_start dispatch costs ~0.6us of sequencer time regardless of size,
    so gathering per group (8 dmas) instead of per chunk (32) keeps the
    sync sequencer free for the output writes. Emitted 1.5 chunks ahead of
    the group's first matmul; sources (z1h/R of that group) are computed a
    full group earlier, so the dispatch never blocks the ring.
    """
    HF = P // 2               # 64 elements per interleaved half
    ZI = p.zi.tile([P, HF * D], f16, tag="ZI", name=f"ZI_{grp}")
    RI = p.ri.tile([P, HF * KD], f16, tag="RI", name=f"RI_{grp}")
    z1h, z2h, R0, R1 = st["z1h"], st["z2h"], st["R0"], st["R1"]
    # group 0 rides the (empty) sync HWDGE ring for the fastest ramp; later
    # groups go via GPSIMD SWDGE where a wait on R can't block the outputs
    dmae = nc.sync if grp == 0 else nc.gpsimd
    for half in range(2):
        hb = HF * half
        pp = 32 * half
        dmae.dma_start(ZI[pp:pp + 1, :], z1h[hb:hb + HF, :])
        dmae.dma_start(ZI[pp + 1:pp + 2, :], z2h[hb:hb + HF, :])
        dmae.dma_start(RI[pp:pp + 1, :], R0[hb:hb + HF, :])
        dmae.dma_start(RI[pp + 1:pp + 2, :], R1[hb:hb + HF, :])
    return ZI, RI


def _emit_chunk(p, nc, ZI, RI, out_hw, grp, ch, qctr):
    """G elements (matmul + quad PSUM->fp16 copy) + output DMA."""
    e0 = grp * P + ch * G     # global element base for this chunk
    ci = grp * NCH + ch
    GS = 8 if ci == 0 else 16
    for sub in range(G // GS):
        STG = p.stage.tile([P, GS * KD], i8, tag="STG",
                           name=f"STG_{grp}_{ch}_{sub}")
        for t in range(GS // QB):
            pt = p.mmp.tile([P, QB * 512], f32, tag="pt",
                            name=f"pt_{grp}_{ch}_{sub}_{t}")
            for slot in range(QB):
                s = sub * GS + t * QB + slot   # local element in chunk
                half = s % 2                   # PE quadrant alternation
                ff = ch * (G // 2) + s // 2    # column within group tile
                pp = 32 * half
                lhsT = ZI[pp:pp + 2, ff * D:(ff + 1) * D]
                rhs = RI[pp:pp + 2, ff * KD:(ff + 1) * KD]
                nc.tensor.matmul(pt[:, slot * 512:slot * 512 + KD],
                                 lhsT, rhs, start=True, stop=True)
            src = pt[:].rearrange("p (q c) -> p q c", c=512)[:, :, 0:KD]
            dst = STG[:, t * QB * KD:(t + 1) * QB * KD].rearrange(
                "p (q c) -> p q c", c=KD)
            if qctr[0] % 2 == 0:
                nc.vector.tensor_copy(dst, src)
            else:
                nc.scalar.copy(dst, src)
            qctr[0] += 1
        es = e0 + sub * GS
        nc.sync.dma_start(out_hw[:, es * KD:(es + GS) * KD], STG[:])


def _build_body(ctx, tc, z1, z2, out_hw, diag_hw, scl_hw):
    nc = tc.nc
    p = _make_pools(ctx, tc)
    p.scl_hw = scl_hw
    p.zero1 = p.const.tile([P, 1], f32, tag="zero1", name="zero1")
    nc.vector.memset(p.zero1[:], 0.0)

    # Stats run TWO groups ahead of consumption (phase0 of g+2 during group
    # g; phases 1-2 of g+1 at the start of group g) so the ~25us stats chain
    # latency (GPS serial ops + cross-engine hops queued behind copies)
    # hides under ~70us of compute. Stats phases are emitted BEFORE the
    # chunk body so their DVE/ACT hops enqueue ahead of that chunk's copies.
    qctr = [0]
    zs = _load_all_z(p, nc, z1, z2)
    sts = {0: _stats_phase0(p, nc, zs, 0)}
    _stats_phase1(p, nc, sts[0], 0)
    gath = {0: _emit_group_gathers(p, nc, sts[0], 0)}
    _stats_phase2(p, nc, sts[0], 0, diag_hw)
    if GROUPS > 1:
        # group 1 also runs on DVE, in the ramp window before copies
        # saturate it, so R(g1) is ready long before the first boundary
        sts[1] = _stats_phase0(p, nc, zs, 1)
        _stats_phase1(p, nc, sts[1], 1)
        _stats_phase2(p, nc, sts[1], 1, diag_hw)
    TOT = GROUPS * NCH
    for ci in range(TOT):
        grp, ch = divmod(ci, NCH)
        if ch == 2 and grp + 1 < GROUPS:
            # next group's gathers: R/z1h computed by now, so the sync
            # dispatch is wait-free, 1.5 chunks ahead of first use
            gath[grp + 1] = _emit_group_gathers(p, nc, sts[grp + 1], grp + 1)
        ZI, RI = gath[grp]
        _emit_chunk(p, nc, ZI, RI, out_hw, grp, ch, qctr)
        # stats for group g+2 go AFTER the chunk body: their DVE/ACT hops
        # then queue behind this chunk's copies instead of blocking them
        # while waiting on the GPS chain; the 2-group prefetch absorbs the
        # added latency
        if grp + 2 < GROUPS:
            if ch == 0:
                sts[grp + 2] = _stats_phase0(p, nc, zs, grp + 2)
            elif ch == 1:
                _stats_phase1(p, nc, sts[grp + 2], grp + 2)
            elif ch == 2:
                _stats_phase2(p, nc, sts[grp + 2], grp + 2, diag_hw)


def build_kernel():
    nc = bacc.Bacc("TRN2", target_bir_lowering=False, debug=False)
    z1 = nc.dram_tensor("z1", [B_SH, D], f32, kind="ExternalInput").ap()
    z2 = nc.dram_tensor("z2", [B_SH, D], f32, kind="ExternalInput").ap()
    # packed rank-2 output: [i partition, (b, k, j) free], fp16
    out_hw = nc.dram_tensor("out", [P, B_SH * KD], i8,
                            kind="ExternalOutput").ap()
    scl_hw = nc.dram_tensor("scl", [GROUPS, P, 1], f32,
                            kind="ExternalOutput").ap()
    # final diagonals: [group, b-partition (interleaved), (k, i) free], f32
    diag_hw = nc.dram_tensor("diag", [GROUPS, P, KD], f32,
                             kind="ExternalOutput").ap()
    with tile.TileContext(nc) as tc:
        with ExitStack() as ctx:
            _build_body(ctx, tc, z1, z2, out_hw, diag_hw, scl_hw)
    nc.compile()
    return nc


_NC_CACHE = None


def _get_nc():
    global _NC_CACHE
    if _NC_CACHE is None:
        _NC_CACHE = build_kernel()
    return _NC_CACHE


def _assemble(out_c, diag_c, scl_c, dst):
    """Unpack one core's HW tensors into dst [3, B_SH, D, D] f32."""
    # out_c [128 i, B_SH*384] int8 -> [i, b, k, j] -> [k, b, i, j]; dequant
    # with the per-element scale bound/126 (scl rows are partition-ordered)
    h = out_c.reshape(P, B_SH, 3, D).transpose(2, 1, 0, 3)
    scale = (scl_c.reshape(GROUPS, P)[:, PART_OF_ELEM].reshape(B_SH)
             / np.float32(126.0))
    np.multiply(h, scale[None, :, None, None], out=dst, dtype=np.float32)
    # diag_c [GROUPS, 128 p, 3*128] f32; partition p holds group element
    # E_OF_P... i.e. ordered element e sits at partition P_OF_E[e]
    dv = diag_c[:, PART_OF_ELEM, :].reshape(GROUPS, P, 3, D).transpose(
        2, 0, 1, 3).reshape(3, B_SH, D)
    dst.reshape(3, B_SH, D * D)[:, :, ::D + 1] = dv


def kernel(z1, z2):
    nc = _get_nc()
    z1 = np.ascontiguousarray(np.asarray(z1, dtype=np.float32))
    z2 = np.ascontiguousarray(np.asarray(z2, dtype=np.float32))
    in_maps = [
        {"z1": z1[c * B_SH:(c + 1) * B_SH], "z2": z2[c * B_SH:(c + 1) * B_SH]}
        for c in range(N_CORES)
    ]
    res = run_bass_kernel_spmd(nc, in_maps, core_ids=list(range(N_CORES)))
    full = np.empty((3, B_FULL, D, D), dtype=np.float32)
    for c in range(N_CORES):
        _assemble(res.results[c]["out"], res.results[c]["diag"],
                  res.results[c]["scl"], full[:, c * B_SH:(c + 1) * B_SH])
    return full
